# revision 1
# baseline (speedup 1.0000x reference)
"""DeepseekV3 MLA attention prefill (S=1024, H=128 heads, HID=7168) on 8 TRN2
NeuronCores.

Sharding: tensor-parallel over heads (16 heads/core); the low-rank input
projections (q_a / kv_a) are sequence-sharded (128 rows/core) and exchanged
with one AllGather of the rms-normed activations (natural [s, col] layout so
the collective moves 4KB rows). Each core emits a partial output projection
(contraction over its own 16 heads, transposed [HID, S] layout so each ow
stationary tile serves two matmuls); the host sums the 8 partials.

All matmul operands are bf16 (weights pre-cast on host, activations cast at
the psum->SBUF copy); softmax/rmsnorm math stays f32. The attention inner
loop is software-pipelined: AV matmuls run one kc-step behind the score
matmuls and the softmax normalization matmuls are deferred into the next
qt/group's instruction stream so exp/mask/reduce latency never stalls the
in-order PE queue. Post-AllGather stitching uses DMA-engine transposes.
"""
import math
import numpy as np
import ml_dtypes

import concourse.bass as bass
import concourse.mybir as mybir
import concourse.bacc as bacc
import concourse.tile as tile
import concourse.bass_utils as bass_utils
from concourse.masks import make_identity
from contextlib import ExitStack

F32 = mybir.dt.float32
BF16 = mybir.dt.bfloat16
AF = mybir.ActivationFunctionType
OP = mybir.AluOpType

N_CORES = 8
S = 1024
HID = 7168
H = 128
HG = H // N_CORES          # 16 heads per core
D_NOPE = 128
D_ROPE = 64
D_Q = D_NOPE + D_ROPE      # 192
D_V = 128
CQ = 1536                  # q lora rank
CKV = 512                  # kv lora rank
CA = CQ + CKV + D_ROPE     # 2112 fused a-proj cols
S_SH = S // N_CORES        # 128 sequence rows per core
CC_A = HID // 128          # 56 contraction chunks for a-proj
NT_A = [(0, 512), (512, 512), (1024, 512), (1536, 512), (2048, 64)]
SCALE = 1.0 / math.sqrt(D_Q)
EPS = 1e-6
G_HEADS = 2                # heads per group
N_GROUPS = HG // G_HEADS   # 8 groups
LAST_EXEC_NS = None

_CACHE = {}


def _dma_rows_to_3d(nc, dst, src_ap, n_chunks, p=128):
    """dst [p, n_chunks, w] <- src rows laid out as (chunk, p)."""
    try:
        nc.sync.dma_start(dst, src_ap.rearrange("(c p) s -> p c s", p=p))
    except Exception:
        for c in range(n_chunks):
            nc.sync.dma_start(dst[:, c, :], src_ap[c * p:(c + 1) * p, :])


def _build_nc():
    nc = bacc.Bacc("TRN2", target_bir_lowering=False, debug=False,
                   num_devices=N_CORES)

    xT = nc.dram_tensor("xT", [HID, S_SH], BF16, kind="ExternalInput")
    wa = nc.dram_tensor("wa", [HID, CA], BF16, kind="ExternalInput")
    qbn = nc.dram_tensor("qbn", [CQ, HG * D_NOPE], BF16, kind="ExternalInput")
    qbp = nc.dram_tensor("qbp", [CQ, HG * D_ROPE], BF16, kind="ExternalInput")
    kvbk = nc.dram_tensor("kvbk", [CKV, HG * D_NOPE], BF16, kind="ExternalInput")
    kvbv = nc.dram_tensor("kvbv", [CKV, HG * D_V], BF16, kind="ExternalInput")
    ow = nc.dram_tensor("ow", [HG * D_V, HID], BF16, kind="ExternalInput")
    cos_s = nc.dram_tensor("cos_s", [S_SH, D_ROPE], F32, kind="ExternalInput")
    sin_sg = nc.dram_tensor("sin_sg", [S_SH, D_ROPE], F32, kind="ExternalInput")
    cos2t = nc.dram_tensor("cos2t", [128, S], F32, kind="ExternalInput")
    sin2tg = nc.dram_tensor("sin2tg", [128, S], F32, kind="ExternalInput")
    masks = nc.dram_tensor("masks", [512, 512], BF16, kind="ExternalInput")
    ones_col = nc.dram_tensor("ones_col", [128, 1], BF16, kind="ExternalInput")
    ones_row = nc.dram_tensor("ones_row", [1, 128], BF16, kind="ExternalInput")
    zeros64 = nc.dram_tensor("zeros64", [64, S], BF16, kind="ExternalInput")
    outT = nc.dram_tensor("outT", [HID, S], BF16, kind="ExternalOutput")

    with tile.TileContext(nc) as tc, ExitStack() as top:
        const = top.enter_context(tc.tile_pool(name="const", bufs=1))
        dram = top.enter_context(tc.tile_pool(name="dram", bufs=1, space="DRAM"))
        outsp = top.enter_context(tc.tile_pool(name="outsp", bufs=1))
        # phase B/C weight + staging pools opened at top level so their
        # prefetch DMAs can be emitted before the AllGather
        sbwq = top.enter_context(tc.tile_pool(name="sbwq", bufs=2))
        sbow = top.enter_context(tc.tile_pool(name="sbow", bufs=2))

        # ---- constants in SBUF ----
        ident = const.tile([128, 128], F32, tag="ident")
        make_identity(nc, ident[:])
        masks_sb = const.tile([128, 4, 512], BF16, tag="masks")
        _dma_rows_to_3d(nc, masks_sb[:], masks.ap(), 4)
        cos_s_sb = const.tile([S_SH, D_ROPE], F32, tag="coss")
        sin_sg_sb = const.tile([S_SH, D_ROPE], F32, tag="sinsg")
        nc.sync.dma_start(cos_s_sb[:], cos_s.ap())
        nc.sync.dma_start(sin_sg_sb[:], sin_sg.ap())
        cos2t_sb = const.tile([128, S], F32, tag="cos2t")
        sin2tg_sb = const.tile([128, S], F32, tag="sin2tg")
        nc.sync.dma_start(cos2t_sb[:], cos2t.ap())
        nc.sync.dma_start(sin2tg_sb[:], sin2tg.ap())
        ones_col_sb = const.tile([128, 1], BF16, tag="onesc")
        ones_row_sb = const.tile([1, 128], BF16, tag="onesr")
        nc.sync.dma_start(ones_col_sb[:], ones_col.ap())
        nc.sync.dma_start(ones_row_sb[:], ones_row.ap())

        agi1 = dram.tile([CQ, S_SH], BF16, tag="agi1")
        ago1 = dram.tile([CQ * N_CORES, S_SH], BF16, tag="ago1", addr_space="Shared")
        agi2 = dram.tile([CA - CQ, S_SH], BF16, tag="agi2")
        ago2 = dram.tile([(CA - CQ) * N_CORES, S_SH], BF16, tag="ago2",
                         addr_space="Shared")

        # all 16 heads' attention outputs live in SBUF [dv=128, head, s]
        outs_sb = outsp.tile([128, HG, S], BF16, tag="outs")

        def load_group_weights(g):
            h0 = g * G_HEADS
            qbnw = sbwq.tile([128, CQ // 128, G_HEADS * 128], BF16,
                             tag="qbnw", name="qbnw")
            qbpw = sbwq.tile([128, CQ // 128, G_HEADS * 64], BF16,
                             tag="qbpw", name="qbpw")
            kvbkw = sbwq.tile([128, CKV // 128, G_HEADS * 128], BF16,
                              tag="kvbkw", name="kvbkw")
            kvbvw = sbwq.tile([128, CKV // 128, G_HEADS * 128], BF16,
                              tag="kvbvw", name="kvbvw")
            _dma_rows_to_3d(nc, qbnw[:],
                            qbn.ap()[:, h0 * 128:(h0 + G_HEADS) * 128], CQ // 128)
            _dma_rows_to_3d(nc, qbpw[:],
                            qbp.ap()[:, h0 * 64:(h0 + G_HEADS) * 64], CQ // 128)
            _dma_rows_to_3d(nc, kvbkw[:],
                            kvbk.ap()[:, h0 * 128:(h0 + G_HEADS) * 128], CKV // 128)
            _dma_rows_to_3d(nc, kvbvw[:],
                            kvbv.ap()[:, h0 * 128:(h0 + G_HEADS) * 128], CKV // 128)
            return qbnw, qbpw, kvbkw, kvbvw

        def load_ow(nt):
            owt_a = sbow.tile([128, 8, 512], BF16, tag="owa", name="owt_a")
            owt_b = sbow.tile([128, 8, 512], BF16, tag="owb", name="owt_b")
            _dma_rows_to_3d(nc, owt_a[:],
                            ow.ap()[0:8 * 128, nt * 512:(nt + 1) * 512], 8)
            _dma_rows_to_3d(nc, owt_b[:],
                            ow.ap()[8 * 128:16 * 128, nt * 512:(nt + 1) * 512], 8)
            return owt_a, owt_b

        # ================= Phase A: fused a-proj + rmsnorm + kpe rope ======
        with ExitStack() as pa:
            sba = pa.enter_context(tc.tile_pool(name="sba", bufs=1))
            sbw = pa.enter_context(tc.tile_pool(name="sbw", bufs=7))
            sbt = pa.enter_context(tc.tile_pool(name="sbt", bufs=2))
            psa = pa.enter_context(tc.tile_pool(name="psa", bufs=1, space="PSUM"))

            xT_sb = sba.tile([128, CC_A, S_SH], BF16, tag="xT")
            for c in range(4):
                nc.sync.dma_start(xT_sb[:, c, :], xT.ap()[c * 128:(c + 1) * 128, :])
            for c0 in range(4, CC_A, 13):
                n = min(13, CC_A - c0)
                _dma_rows_to_3d(nc, xT_sb[:, c0:c0 + n, :],
                                xT.ap()[c0 * 128:(c0 + n) * 128, :], n)
            acts = sba.tile([S_SH, CA], F32, tag="acts")

            pa_ps = [psa.tile([128, 512], F32, tag="a0", name="pa0"),
                     psa.tile([128, 512], F32, tag="a1", name="pa1"),
                     psa.tile([128, 512], F32, tag="a2", name="pa2"),
                     psa.tile([128, 512], F32, tag="a3", name="pa3"),
                     psa.tile([128, 64], F32, tag="a4", name="pa4")]
            # pass 1: q_c columns 0:1536
            for cc in range(CC_A):
                wt = sbw.tile([128, CQ], BF16, tag="wa1", name="wt")
                nc.sync.dma_start(wt[:, 0:768],
                                  wa.ap()[cc * 128:(cc + 1) * 128, 0:768])
                nc.sync.dma_start(wt[:, 768:CQ],
                                  wa.ap()[cc * 128:(cc + 1) * 128, 768:CQ])
                for j in range(3):
                    nc.tensor.matmul(pa_ps[j][:], xT_sb[:, cc, :],
                                     wt[:, j * 512:(j + 1) * 512],
                                     start=(cc == 0), stop=(cc == CC_A - 1))
            for j in range(3):
                nc.scalar.copy(acts[:, j * 512:(j + 1) * 512], pa_ps[j][:])

            # q rmsnorm + transpose chunks 0:12 -> agi1
            sq = sba.tile([S_SH, CQ + CKV], F32, tag="sq")
            fq = sbt.tile([S_SH, 1], F32, tag="fq")
            fk = sbt.tile([S_SH, 1], F32, tag="fk")
            nc.vector.tensor_mul(sq[:, 0:CQ], acts[:, 0:CQ], acts[:, 0:CQ])
            nc.vector.reduce_sum(fq[:], sq[:, 0:CQ], axis=mybir.AxisListType.X)
            nc.vector.tensor_scalar(fq[:], fq[:], 1.0 / CQ, EPS, OP.mult, OP.add)
            nc.vector.reciprocal(fq[:], fq[:])
            nc.scalar.activation(fq[:], fq[:], AF.Sqrt)
            nc.vector.tensor_scalar_mul(acts[:, 0:CQ], acts[:, 0:CQ], fq[:])

            bT = sba.tile([128, 17 * 128], BF16, tag="bT")
            for t in range(12):
                pt_ps = psa.tile([128, 128], F32, tag="tp", name="pt_ps")
                nc.tensor.transpose(pt_ps[:, 0:128], acts[:, t * 128:(t + 1) * 128], ident[:])
                nc.scalar.copy(bT[:, t * 128:(t + 1) * 128], pt_ps[:, 0:128])
                nc.sync.dma_start(agi1[t * 128:(t + 1) * 128, :], bT[:, t * 128:(t + 1) * 128])

            nc.gpsimd.collective_compute(
                "AllGather", OP.bypass,
                replica_groups=[list(range(N_CORES))],
                ins=[agi1.opt()], outs=[ago1.opt()],
            )

            # pass 2: ckv + k_pe columns 1536:2112
            for cc in range(CC_A):
                wt = sbw.tile([128, CA - CQ], BF16, tag="wa2", name="wt")
                nc.sync.dma_start(wt[:], wa.ap()[cc * 128:(cc + 1) * 128, CQ:CA])
                nc.tensor.matmul(pa_ps[3][:], xT_sb[:, cc, :], wt[:, 0:512],
                                 start=(cc == 0), stop=(cc == CC_A - 1))
                nc.tensor.matmul(pa_ps[4][:], xT_sb[:, cc, :], wt[:, 512:576],
                                 start=(cc == 0), stop=(cc == CC_A - 1))
            nc.scalar.copy(acts[:, CQ:CQ + 512], pa_ps[3][:])
            nc.scalar.copy(acts[:, CQ + 512:CA], pa_ps[4][:, 0:64])

            nc.vector.tensor_mul(sq[:, CQ:CQ + CKV], acts[:, CQ:CQ + CKV],
                                 acts[:, CQ:CQ + CKV])
            nc.vector.reduce_sum(fk[:], sq[:, CQ:CQ + CKV], axis=mybir.AxisListType.X)
            nc.vector.tensor_scalar(fk[:], fk[:], 1.0 / CKV, EPS, OP.mult, OP.add)
            nc.vector.reciprocal(fk[:], fk[:])
            nc.scalar.activation(fk[:], fk[:], AF.Sqrt)
            nc.vector.tensor_scalar_mul(acts[:, CQ:CQ + CKV], acts[:, CQ:CQ + CKV], fk[:])

            # k_pe rope (natural [s, 64] layout), cols 2048:2112
            kp0 = CQ + CKV
            kv1 = sbt.tile([S_SH, D_ROPE], F32, tag="kv1")
            kv2 = sbt.tile([S_SH, D_ROPE], F32, tag="kv2")
            nc.vector.tensor_mul(kv1[:], acts[:, kp0:kp0 + 64], cos_s_sb[:])
            nc.vector.tensor_mul(kv2[:, 0:32], acts[:, kp0 + 32:kp0 + 64], sin_sg_sb[:, 0:32])
            nc.vector.tensor_mul(kv2[:, 32:64], acts[:, kp0:kp0 + 32], sin_sg_sb[:, 32:64])
            nc.vector.tensor_add(acts[:, kp0:kp0 + 64], kv1[:], kv2[:])

            for t in range(12, 17):
                w = 128 if t < 16 else 64
                pt_ps = psa.tile([128, 128], F32, tag="tp", name="pt_ps")
                nc.tensor.transpose(pt_ps[:w, 0:128], acts[:, t * 128:t * 128 + w], ident[:])
                nc.scalar.copy(bT[:w, t * 128:(t + 1) * 128], pt_ps[:w, 0:128])
                nc.sync.dma_start(agi2[(t - 12) * 128:(t - 12) * 128 + w, :],
                                  bT[:w, t * 128:(t + 1) * 128])

        # prefetch first two groups' weights + first ow tile before the AG
        # so the DMA queues aren't blocked behind AG-dependent transfers
        wq_pre = [load_group_weights(0), load_group_weights(1)]
        ow_pre = load_ow(0)

        nc.gpsimd.collective_compute(
            "AllGather", OP.bypass,
            replica_groups=[list(range(N_CORES))],
            ins=[agi2.opt()], outs=[ago2.opt()],
        )

        # ================= Phase B: per-head-group projections + attention ==
        with ExitStack() as pb:
            sbg = pb.enter_context(tc.tile_pool(name="sbg", bufs=1))
            sbh = pb.enter_context(tc.tile_pool(name="sbh", bufs=2))
            sbp = pb.enter_context(tc.tile_pool(name="sbp", bufs=1))
            sbv = pb.enter_context(tc.tile_pool(name="sbv", bufs=2))
            sbpt = pb.enter_context(tc.tile_pool(name="sbpt", bufs=5))
            sbs = pb.enter_context(tc.tile_pool(name="sbs", bufs=2))
            sbo = pb.enter_context(tc.tile_pool(name="sbo", bufs=3))
            ps_main = pb.enter_context(tc.tile_pool(name="ps_main", bufs=3, space="PSUM"))
            ps_o = pb.enter_context(tc.tile_pool(name="ps_o", bufs=2, space="PSUM"))
            ps_sm = pb.enter_context(tc.tile_pool(name="ps_sm", bufs=1, space="PSUM"))

            # stitch gathered activations, per 512-wide s-tile
            qct = []
            ckv = []
            for st in range(2):
                q_t = sbg.tile([128, CQ // 128, 512], BF16, tag=f"qct{st}",
                               name=f"qct{st}")
                k_t = sbg.tile([128, CKV // 128, 512], BF16, tag=f"ckv{st}",
                               name=f"ckv{st}")
                for r in range(4):
                    core = st * 4 + r
                    _dma_rows_to_3d(nc, q_t[:, :, r * 128:(r + 1) * 128],
                                    ago1[core * CQ:(core + 1) * CQ, :], CQ // 128)
                    _dma_rows_to_3d(nc, k_t[:, :, r * 128:(r + 1) * 128],
                                    ago2[core * 576:core * 576 + CKV, :], CKV // 128)
                qct.append(q_t)
                ckv.append(k_t)
            # zero-padded per-head k_pe tiles so the rope matmuls load a full
            # 128-row stationary (fast weight load needs 128)
            kpeA = sbg.tile([128, S], BF16, tag="kpeA")
            kpeB = sbg.tile([128, S], BF16, tag="kpeB")
            nc.sync.dma_start(kpeA[64:128, :], zeros64.ap())
            nc.sync.dma_start(kpeB[0:64, :], zeros64.ap())
            for core in range(N_CORES):
                base = core * 576 + CKV
                nc.sync.dma_start(kpeA[0:64, core * 128:(core + 1) * 128],
                                  ago2[base:base + 64, :])
                nc.sync.dma_start(kpeB[64:128, core * 128:(core + 1) * 128],
                                  ago2[base:base + 64, :])

            pending_norm = []

            def emit_normalize():
                while pending_norm:
                    h_idx, qt_, sums_, psum_o_ = pending_norm.pop(0)
                    sums_b = sbpt.tile([128, 512], BF16, tag="sums_b",
                                       name="sums_b")
                    nc.vector.tensor_copy(sums_b[:], sums_[:])
                    pss = ps_sm.tile([128, 512], F32, tag="pss", name="pss")
                    nc.tensor.matmul(pss[0:1, :], ones_col_sb[:], sums_b[:],
                                     start=True, stop=True)
                    rec = sbs.tile([1, 512], BF16, tag="rec", name="rec")
                    with nc.allow_low_precision(reason="softmax recip in bf16"):
                        nc.vector.reciprocal(rec[:], pss[0:1, :])
                    psb = ps_sm.tile([128, 512], F32, tag="pss", name="psb")
                    nc.tensor.matmul(psb[:], ones_row_sb[:], rec[:],
                                     start=True, stop=True)
                    bsb = sbs.tile([128, 512], F32, tag="bsb", name="bsb")
                    nc.vector.tensor_copy(bsb[:], psb[:])
                    nc.vector.tensor_mul(
                        outs_sb[:, h_idx, qt_ * 512:(qt_ + 1) * 512],
                        psum_o_[:], bsb[:])

            for g in range(N_GROUPS):
                h0 = g * G_HEADS
                if g < 2:
                    qbnw, qbpw, kvbkw, kvbvw = wq_pre[g]
                else:
                    qbnw, qbpw, kvbkw, kvbvw = load_group_weights(g)

                # --- projections; st-paired so each stationary is reused ---
                qTn = []
                kTn = []
                for i in range(G_HEADS):
                    qt_t = sbh.tile([128, S], BF16, tag=f"qTn{i}", name=f"qTn{i}")
                    _ = qt_t
                    p0 = ps_main.tile([128, 512], F32, tag="s", name="p0")
                    p1 = ps_main.tile([128, 512], F32, tag="s", name="p1")
                    if g == 0 and i == 0:
                        # first chains after the AllGather: unpaired, so the
                        # st=0 chain starts as soon as half the stitch lands
                        for c in range(CQ // 128):
                            nc.tensor.matmul(p0[:], qbnw[:, c, 0:128],
                                             qct[0][:, c, :],
                                             start=(c == 0), stop=(c == CQ // 128 - 1))
                        for c in range(CQ // 128):
                            nc.tensor.matmul(p1[:], qbnw[:, c, 0:128],
                                             qct[1][:, c, :],
                                             start=(c == 0), stop=(c == CQ // 128 - 1))
                    else:
                        for c in range(CQ // 128):
                            nc.tensor.matmul(p0[:], qbnw[:, c, i * 128:(i + 1) * 128],
                                             qct[0][:, c, :],
                                             start=(c == 0), stop=(c == CQ // 128 - 1))
                            nc.tensor.matmul(p1[:], qbnw[:, c, i * 128:(i + 1) * 128],
                                             qct[1][:, c, :],
                                             start=(c == 0), stop=(c == CQ // 128 - 1))
                    nc.scalar.copy(qt_t[:, 0:512], p0[:])
                    nc.scalar.copy(qt_t[:, 512:1024], p1[:])
                    qTn.append(qt_t)
                    if g > 0 or i > 0:
                        emit_normalize()
                qp_raw = sbp.tile([128, S], F32, tag="qp_raw", name="qp_raw")
                p0 = ps_main.tile([128, 512], F32, tag="s", name="p0")
                p1 = ps_main.tile([128, 512], F32, tag="s", name="p1")
                for c in range(CQ // 128):
                    nc.tensor.matmul(p0[:], qbpw[:, c, :], qct[0][:, c, :],
                                     start=(c == 0), stop=(c == CQ // 128 - 1))
                    nc.tensor.matmul(p1[:], qbpw[:, c, :], qct[1][:, c, :],
                                     start=(c == 0), stop=(c == CQ // 128 - 1))
                nc.scalar.copy(qp_raw[:, 0:512], p0[:])
                nc.scalar.copy(qp_raw[:, 512:1024], p1[:])
                # rope on the head-pair tile: rows [0:64]=head h0, [64:128]=h0+1
                qTp = sbh.tile([128, S], BF16, tag="qTp")
                rm = sbp.tile([128, S], F32, tag="ropem")
                rs = sbp.tile([128, S], F32, tag="ropes")
                for hh in range(2):
                    sl = slice(hh * 512, (hh + 1) * 512)
                    nc.vector.tensor_mul(rm[:, sl], qp_raw[:, sl], cos2t_sb[:, sl])
                    for b in range(4):
                        r0 = b * 32
                        r1 = r0 + 32 if b % 2 == 0 else r0 - 32
                        nc.vector.tensor_copy(rs[r0:r0 + 32, sl], qp_raw[r1:r1 + 32, sl])
                    nc.vector.tensor_mul(rs[:, sl], rs[:, sl], sin2tg_sb[:, sl])
                    nc.vector.tensor_add(qTp[:, sl], rm[:, sl], rs[:, sl])

                for i in range(G_HEADS):
                    kt_t = sbh.tile([128, S], BF16, tag=f"kTn{i}", name=f"kTn{i}")
                    p0 = ps_main.tile([128, 512], F32, tag="s", name="p0")
                    p1 = ps_main.tile([128, 512], F32, tag="s", name="p1")
                    for c in range(CKV // 128):
                        nc.tensor.matmul(p0[:], kvbkw[:, c, i * 128:(i + 1) * 128],
                                         ckv[0][:, c, :],
                                         start=(c == 0), stop=(c == CKV // 128 - 1))
                        nc.tensor.matmul(p1[:], kvbkw[:, c, i * 128:(i + 1) * 128],
                                         ckv[1][:, c, :],
                                         start=(c == 0), stop=(c == CKV // 128 - 1))
                    nc.scalar.copy(kt_t[:, 0:512], p0[:])
                    nc.scalar.copy(kt_t[:, 512:1024], p1[:])
                    kTn.append(kt_t)

                v_g = sbv.tile([128, 8, G_HEADS * 128], BF16, tag="v_g")
                for sc in range(8):
                    st = sc // 4
                    psum = ps_main.tile([128, 512], F32, tag="s", name="psum")
                    nn = G_HEADS * 128
                    for c in range(CKV // 128):
                        nc.tensor.matmul(
                            psum[:, :nn],
                            ckv[st][:, c, (sc % 4) * 128:(sc % 4 + 1) * 128],
                            kvbvw[:, c, :],
                            start=(c == 0), stop=(c == CKV // 128 - 1))
                    nc.scalar.copy(v_g[:, sc, :], psum[:, :nn])

                # --- attention: heads interleaved, AV skewed one kc behind ---
                for qt in range(2):
                    kmax = 4 * (qt + 1)
                    sums = [sbs.tile([128, 512], F32, tag=f"sums{i}", name=f"sums{i}")
                            for i in range(G_HEADS)]
                    psum_o = [ps_o.tile([128, 512], F32, tag=f"o{i}", name=f"po{i}")
                              for i in range(G_HEADS)]
                    pt = {}
                    for kc in range(kmax):
                        for i in range(G_HEADS):
                            ps = ps_main.tile([128, 512], F32, tag="s", name="ps")
                            nc.tensor.matmul(ps[:], kTn[i][:, kc * 128:(kc + 1) * 128],
                                             qTn[i][:, qt * 512:(qt + 1) * 512],
                                             start=True, stop=False)
                            kpe = kpeA if i == 0 else kpeB
                            nc.tensor.matmul(ps[:], kpe[:, kc * 128:(kc + 1) * 128],
                                             qTp[:, qt * 512:(qt + 1) * 512],
                                             start=False, stop=True)
                            p = sbpt.tile([128, 512], BF16, tag="pt", name="p")
                            nc.scalar.activation(p[:], ps[:], AF.Exp, scale=SCALE)
                            if kc >= 4 * qt:
                                nc.vector.tensor_mul(p[:], p[:],
                                                     masks_sb[:, kc - 4 * qt, :])
                            if kc == 0:
                                nc.vector.tensor_copy(sums[i][:], p[:])
                            else:
                                nc.vector.tensor_add(sums[i][:], sums[i][:], p[:])
                            pt[(i, kc)] = p
                        if kc > 0:
                            for i in range(G_HEADS):
                                nc.tensor.matmul(psum_o[i][:],
                                                 v_g[:, kc - 1, i * 128:(i + 1) * 128],
                                                 pt[(i, kc - 1)][:],
                                                 start=(kc == 1), stop=False)
                        if qt == 1 and kc == 1:
                            emit_normalize()   # previous qt's softmax tail
                    for i in range(G_HEADS):
                        nc.tensor.matmul(psum_o[i][:],
                                         v_g[:, kmax - 1, i * 128:(i + 1) * 128],
                                         pt[(i, kmax - 1)][:],
                                         start=(kmax == 1), stop=True)
                    for i in range(G_HEADS):
                        pending_norm.append((h0 + i, qt, sums[i], psum_o[i]))

            emit_normalize()

            # ========= Phase C: partial output projection, out^T layout =====
            # out^T[hid, s] = sum_hc ow[hc]^T @ outs[hc]; each ow stationary
            # slice serves both 512-wide s-blocks before the next LDWEIGHTS.
            for nt in range(HID // 512):
                owt_a, owt_b = ow_pre if nt == 0 else load_ow(nt)
                for ntl in range(4):
                    pA = ps_main.tile([128, 512], F32, tag="s", name="pA")
                    pB = ps_main.tile([128, 512], F32, tag="s", name="pB")
                    for hc in range(HG):
                        owt = owt_a if hc < 8 else owt_b
                        lhs = owt[:, hc % 8, ntl * 128:(ntl + 1) * 128]
                        nc.tensor.matmul(pA[:], lhs, outs_sb[:, hc, 0:512],
                                         start=(hc == 0), stop=(hc == HG - 1))
                        nc.tensor.matmul(pB[:], lhs, outs_sb[:, hc, 512:1024],
                                         start=(hc == 0), stop=(hc == HG - 1))
                    for half, pp in ((0, pA), (1, pB)):
                        osb = sbo.tile([128, 512], BF16, tag="osb", name="osb")
                        nc.scalar.copy(osb[:], pp[:])
                        nc.sync.dma_start(
                            outT.ap()[nt * 512 + ntl * 128:nt * 512 + (ntl + 1) * 128,
                                      half * 512:(half + 1) * 512], osb[:])

    nc.compile()
    return nc


def _host_inputs(hidden_states, position_ids, q_a_weight, q_a_layernorm_weight,
                 q_b_weight, kv_a_weight, kv_a_layernorm_weight, kv_b_weight,
                 o_weight):
    bf16 = ml_dtypes.bfloat16
    x = np.asarray(hidden_states, np.float32).reshape(S, HID)
    pos = np.asarray(position_ids, np.float64).reshape(S)
    q_a_w = np.asarray(q_a_weight, np.float32)
    q_ln = np.asarray(q_a_layernorm_weight, np.float32)
    q_b_w = np.asarray(q_b_weight, np.float32)
    kv_a_w = np.asarray(kv_a_weight, np.float32)
    kv_ln = np.asarray(kv_a_layernorm_weight, np.float32)
    kv_b_w = np.asarray(kv_b_weight, np.float32)
    o_w = np.asarray(o_weight, np.float32)

    wa = np.concatenate([q_a_w, kv_a_w], axis=1).astype(bf16)  # [HID, 2112]
    xT = np.ascontiguousarray(x.T).astype(bf16)                # [HID, S]

    # fold the rms-norm weights into the b-projections
    qb = (q_ln[:, None] * q_b_w).reshape(CQ, H, D_Q)
    kvb = (kv_ln[:, None] * kv_b_w).reshape(CKV, H, D_NOPE + D_V)

    # rope tables
    inv_freq = 1.0 / (10000.0 ** (np.arange(0, D_ROPE, 2, dtype=np.float64) / D_ROPE))
    freqs = pos[:, None] * inv_freq[None, :]                # [S, 32]
    emb = np.concatenate([freqs, freqs], axis=-1)           # [S, 64]
    cos = np.cos(emb).astype(np.float32)
    sin = np.sin(emb).astype(np.float32)
    sin_sg = np.concatenate([-sin[:, :32], sin[:, 32:]], axis=1)  # [S, 64]
    cosT = np.ascontiguousarray(cos.T)                      # [64, S]
    sinT_sg = np.ascontiguousarray(sin_sg.T)                # [64, S]
    cos2t = np.concatenate([cosT, cosT], axis=0)            # [128, S]
    sin2tg = np.concatenate([sinT_sg, sinT_sg], axis=0)     # [128, S]

    # causal masks for the 4 diagonal offsets
    masks = np.zeros((4, 128, 512), np.float32)
    i = np.arange(128)[:, None]
    j = np.arange(512)[None, :]
    for m in range(4):
        masks[m] = ((i + m * 128) <= j).astype(np.float32)
    masks = masks.reshape(512, 512).astype(bf16)

    ones_col = np.ones((128, 1), bf16)
    ones_row = np.ones((1, 128), bf16)
    zeros64 = np.zeros((64, S), bf16)

    in_maps = []
    for c in range(N_CORES):
        hs = slice(c * HG, (c + 1) * HG)
        in_maps.append({
            "xT": np.ascontiguousarray(xT[:, c * S_SH:(c + 1) * S_SH]),
            "wa": wa,
            "qbn": np.ascontiguousarray(
                qb[:, hs, :D_NOPE].reshape(CQ, HG * D_NOPE)).astype(bf16),
            "qbp": np.ascontiguousarray(
                qb[:, hs, D_NOPE:].reshape(CQ, HG * D_ROPE)).astype(bf16),
            "kvbk": np.ascontiguousarray(
                kvb[:, hs, :D_NOPE].reshape(CKV, HG * D_NOPE)).astype(bf16),
            "kvbv": np.ascontiguousarray(
                kvb[:, hs, D_NOPE:].reshape(CKV, HG * D_V)).astype(bf16),
            "ow": np.ascontiguousarray(
                o_w[c * HG * D_V:(c + 1) * HG * D_V, :]).astype(bf16),
            "cos_s": np.ascontiguousarray(cos[c * S_SH:(c + 1) * S_SH, :]),
            "sin_sg": np.ascontiguousarray(sin_sg[c * S_SH:(c + 1) * S_SH, :]),
            "cos2t": cos2t,
            "sin2tg": sin2tg,
            "masks": masks,
            "ones_col": ones_col,
            "ones_row": ones_row,
            "zeros64": zeros64,
        })
    return in_maps


def kernel(**inputs):
    global LAST_EXEC_NS
    trace = bool(inputs.pop("_trace", False))
    in_maps = _host_inputs(**inputs)
    if "nc" not in _CACHE:
        _CACHE["nc"] = _build_nc()
    nc = _CACHE["nc"]
    res = bass_utils.run_bass_kernel_spmd(
        nc, in_maps, core_ids=list(range(N_CORES)), trace=trace)
    LAST_EXEC_NS = res.exec_time_ns
    total = np.zeros((HID, S), np.float64)
    for c in range(N_CORES):
        total += res.results[c]["outT"].astype(np.float64)
    return np.ascontiguousarray(total.T).astype(np.float32).reshape(1, 1, S, HID)



# revision 15
# speedup vs baseline: 1.0920x; 1.0920x over previous
"""DeepseekV3 MLA attention prefill (S=1024, H=128 heads, HID=7168) on 8 TRN2
NeuronCores.

Sharding: tensor-parallel over heads (16 heads/core); the low-rank input
projections (q_a / kv_a) are sequence-sharded (128 rows/core) and exchanged
with AllGathers of the rms-normed activations. Each core emits a partial
output projection (contraction over its own 16 heads, transposed [HID, S]
layout); the host sums the 8 partials.

v2 schedule (vs v1):
- Phase A computes the kv columns FIRST and fires AllGather2 early, so the
  ckv stitch + kv_b projections for 6 head-groups overlap the q-column pass
  and AllGather1 (v1 idled the PE 56us at this boundary).
- Softmax normalization no longer touches the PE: denominators are reduced
  across partitions with gpsimd.partition_all_reduce, inverted with the fast
  DVE reciprocal, and multiplied into the output. (v1's ones-matmul +
  1-partition DVE reciprocal chain stalled the in-order PE queue ~9us/group.)
- Fine-grained causal windows: score/exp/AV streams start at the diagonal
  (25% fewer streamed columns + smaller masks).
All matmul operands are bf16; softmax/rmsnorm math stays f32. The attention
inner loop is software-pipelined: AV matmuls run one kc-step behind the score
matmuls; normalization is deferred into the next group's instruction stream.
"""
import math
import numpy as np
import ml_dtypes

import concourse.bass as bass
import concourse.mybir as mybir
import concourse.bacc as bacc
import concourse.bass_isa as bass_isa
import concourse.tile as tile
import concourse.bass_utils as bass_utils
from concourse.masks import make_identity
from contextlib import ExitStack

F32 = mybir.dt.float32
BF16 = mybir.dt.bfloat16
AF = mybir.ActivationFunctionType
OP = mybir.AluOpType
RED = bass_isa.ReduceOp

N_CORES = 8
S = 1024
HID = 7168
H = 128
HG = H // N_CORES          # 16 heads per core
D_NOPE = 128
D_ROPE = 64
D_Q = D_NOPE + D_ROPE      # 192
D_V = 128
CQ = 1536                  # q lora rank
CKV = 512                  # kv lora rank
CA = CQ + CKV + D_ROPE     # 2112 fused a-proj cols
S_SH = S // N_CORES        # 128 sequence rows per core
CC_A = HID // 128          # 56 contraction chunks for a-proj
SCALE = 1.0 / math.sqrt(D_Q)
EPS = 1e-6
G_HEADS = 2                # heads per group
N_GROUPS = HG // G_HEADS   # 8 groups
KV_PRE = 6                 # head-groups whose kv_b is computed before phase B
LAST_EXEC_NS = None

_CACHE = {}


def _dma_rows_to_3d(nc, dst, src_ap, n_chunks, p=128):
    """dst [p, n_chunks, w] <- src rows laid out as (chunk, p)."""
    try:
        nc.sync.dma_start(dst, src_ap.rearrange("(c p) s -> p c s", p=p))
    except Exception:
        for c in range(n_chunks):
            nc.sync.dma_start(dst[:, c, :], src_ap[c * p:(c + 1) * p, :])


def _build_nc():
    nc = bacc.Bacc("TRN2", target_bir_lowering=False, debug=False,
                   num_devices=N_CORES)

    xT = nc.dram_tensor("xT", [HID, S_SH], BF16, kind="ExternalInput")
    wa = nc.dram_tensor("wa", [HID, CA], BF16, kind="ExternalInput")
    qbn = nc.dram_tensor("qbn", [CQ, HG * D_NOPE], BF16, kind="ExternalInput")
    qbp = nc.dram_tensor("qbp", [CQ, HG * D_ROPE], BF16, kind="ExternalInput")
    kvbk = nc.dram_tensor("kvbk", [CKV, HG * D_NOPE], BF16, kind="ExternalInput")
    kvbv = nc.dram_tensor("kvbv", [CKV, HG * D_V], BF16, kind="ExternalInput")
    ow = nc.dram_tensor("ow", [HG * D_V, HID], BF16, kind="ExternalInput")
    cos_s = nc.dram_tensor("cos_s", [S_SH, D_ROPE], F32, kind="ExternalInput")
    sin_sg = nc.dram_tensor("sin_sg", [S_SH, D_ROPE], F32, kind="ExternalInput")
    cos2t = nc.dram_tensor("cos2t", [128, S], F32, kind="ExternalInput")
    sin2tg = nc.dram_tensor("sin2tg", [128, S], F32, kind="ExternalInput")
    tri = nc.dram_tensor("tri", [128, 128], BF16, kind="ExternalInput")
    outT = nc.dram_tensor("outT", [HID, S], BF16, kind="ExternalOutput")

    with tile.TileContext(nc) as tc, ExitStack() as top:
        const = top.enter_context(tc.tile_pool(name="const", bufs=1))
        dram = top.enter_context(tc.tile_pool(name="dram", bufs=1, space="DRAM"))
        outsp = top.enter_context(tc.tile_pool(name="outsp", bufs=1))
        # weight + staging pools opened at top level so prefetch DMAs can be
        # emitted early
        sbwq = top.enter_context(tc.tile_pool(name="sbwq", bufs=2))
        sbow = top.enter_context(tc.tile_pool(name="sbow", bufs=2))
        sbkw = top.enter_context(tc.tile_pool(name="sbkw", bufs=2))
        sbkv = top.enter_context(tc.tile_pool(name="sbkv", bufs=4))
        sbg = top.enter_context(tc.tile_pool(name="sbg", bufs=1))
        ps_main = top.enter_context(tc.tile_pool(name="ps_main", bufs=3,
                                                 space="PSUM"))

        # ---- constants in SBUF ----
        ident = const.tile([128, 128], F32, tag="ident")
        make_identity(nc, ident[:])
        tri_sb = const.tile([128, 128], BF16, tag="tri")
        nc.sync.dma_start(tri_sb[:], tri.ap())
        cos_s_sb = const.tile([S_SH, D_ROPE], F32, tag="coss")
        sin_sg_sb = const.tile([S_SH, D_ROPE], F32, tag="sinsg")
        nc.sync.dma_start(cos_s_sb[:], cos_s.ap())
        nc.sync.dma_start(sin_sg_sb[:], sin_sg.ap())
        cos2t_sb = const.tile([128, S], F32, tag="cos2t")
        sin2tg_sb = const.tile([128, S], F32, tag="sin2tg")
        nc.sync.dma_start(cos2t_sb[:], cos2t.ap())
        nc.sync.dma_start(sin2tg_sb[:], sin2tg.ap())

        agi2 = dram.tile([CA - CQ, S_SH], BF16, tag="agi2")
        ago2 = dram.tile([(CA - CQ) * N_CORES, S_SH], BF16, tag="ago2",
                         addr_space="Shared")
        agi1 = dram.tile([CQ, S_SH], BF16, tag="agi1")
        ago1 = dram.tile([CQ * N_CORES, S_SH], BF16, tag="ago1",
                         addr_space="Shared")

        # all 16 heads' attention outputs live in SBUF [dv=128, head, s]
        outs_sb = outsp.tile([128, HG, S], BF16, tag="outs")

        def load_group_qw(g):
            h0 = g * G_HEADS
            qbnw = sbwq.tile([128, CQ // 128, G_HEADS * 128], BF16,
                             tag="qbnw", name="qbnw")
            qbpw = sbwq.tile([128, CQ // 128, G_HEADS * 64], BF16,
                             tag="qbpw", name="qbpw")
            _dma_rows_to_3d(nc, qbnw[:],
                            qbn.ap()[:, h0 * 128:(h0 + G_HEADS) * 128], CQ // 128)
            _dma_rows_to_3d(nc, qbpw[:],
                            qbp.ap()[:, h0 * 64:(h0 + G_HEADS) * 64], CQ // 128)
            return qbnw, qbpw

        def load_group_kvw(g):
            h0 = g * G_HEADS
            kvbkw = sbkw.tile([128, CKV // 128, G_HEADS * 128], BF16,
                              tag="kvbkw", name="kvbkw")
            kvbvw = sbkw.tile([128, CKV // 128, G_HEADS * 128], BF16,
                              tag="kvbvw", name="kvbvw")
            _dma_rows_to_3d(nc, kvbkw[:],
                            kvbk.ap()[:, h0 * 128:(h0 + G_HEADS) * 128], CKV // 128)
            _dma_rows_to_3d(nc, kvbvw[:],
                            kvbv.ap()[:, h0 * 128:(h0 + G_HEADS) * 128], CKV // 128)
            return kvbkw, kvbvw

        def load_ow(nt):
            owt = sbow.tile([128, HG, 256], BF16, tag="owt", name="owt")
            _dma_rows_to_3d(nc, owt[:],
                            ow.ap()[:, nt * 256:(nt + 1) * 256], HG)
            return owt

        # kv_b output tiles: kT [128(dk), head, S], v [128(s), sc, 2*128(dv)].
        # 4-slot sliding window: kv for group g+3 is produced while group g
        # attends, so at most 4 groups' tiles are ever live.
        kv_tiles = {}

        def emit_kvb(g, kvbkw, kvbvw, ckv):
            kT_g = sbkv.tile([128, G_HEADS, S], BF16, tag="kT", name=f"kT{g}")
            v_g = sbkv.tile([128, 8, G_HEADS * 128], BF16, tag="v",
                            name=f"v{g}")
            kv_tiles[g] = (kT_g, v_g)
            for i in range(G_HEADS):
                for st in range(2):
                    p = ps_main.tile([128, 512], F32, tag="s", name="pkv")
                    for c in range(CKV // 128):
                        nc.tensor.matmul(p[:], kvbkw[:, c, i * 128:(i + 1) * 128],
                                         ckv[st][:, c, :],
                                         start=(c == 0), stop=(c == CKV // 128 - 1))
                    nc.scalar.copy(kT_g[:, i, st * 512:(st + 1) * 512], p[:])
            for sc in range(8):
                st = sc // 4
                p = ps_main.tile([128, 512], F32, tag="s", name="pkv")
                nn = G_HEADS * 128
                for c in range(CKV // 128):
                    nc.tensor.matmul(
                        p[:, :nn],
                        ckv[st][:, c, (sc % 4) * 128:(sc % 4 + 1) * 128],
                        kvbvw[:, c, :],
                        start=(c == 0), stop=(c == CKV // 128 - 1))
                nc.scalar.copy(v_g[:, sc, :], p[:, :nn])

        # ================= Phase A =========================================
        with ExitStack() as pa:
            sba = pa.enter_context(tc.tile_pool(name="sba", bufs=1))
            sbw = pa.enter_context(tc.tile_pool(name="sbw", bufs=4))
            sbt = pa.enter_context(tc.tile_pool(name="sbt", bufs=2))
            psa = pa.enter_context(tc.tile_pool(name="psa", bufs=1, space="PSUM"))

            xT_sb = sba.tile([128, CC_A, S_SH], BF16, tag="xT")
            for c in range(4):
                nc.sync.dma_start(xT_sb[:, c, :], xT.ap()[c * 128:(c + 1) * 128, :])
            for c0 in range(4, CC_A, 13):
                n = min(13, CC_A - c0)
                _dma_rows_to_3d(nc, xT_sb[:, c0:c0 + n, :],
                                xT.ap()[c0 * 128:(c0 + n) * 128, :], n)
            acts = sba.tile([S_SH, CA], F32, tag="acts")
            sqs = sba.tile([S_SH, 512], F32, tag="sqs")
            bT = sba.tile([128, 17 * 128], BF16, tag="bT")

            # ---- pass 2 FIRST: ckv + k_pe columns 1536:2112 ----
            p20 = psa.tile([128, 512], F32, tag="a0", name="p20")
            p21 = psa.tile([128, 64], F32, tag="a1", name="p21")
            for cc in range(CC_A):
                wt = sbw.tile([128, CA - CQ], BF16, tag="wa2", name="wt")
                nc.sync.dma_start(wt[:], wa.ap()[cc * 128:(cc + 1) * 128, CQ:CA])
                nc.tensor.matmul(p20[:], xT_sb[:, cc, :], wt[:, 0:512],
                                 start=(cc == 0), stop=(cc == CC_A - 1))
                nc.tensor.matmul(p21[:], xT_sb[:, cc, :], wt[:, 512:576],
                                 start=(cc == 0), stop=(cc == CC_A - 1))
            nc.scalar.copy(acts[:, CQ:CQ + 512], p20[:])
            nc.scalar.copy(acts[:, CQ + 512:CA], p21[:, 0:64])

            fk = sbt.tile([S_SH, 1], F32, tag="fk")
            nc.vector.tensor_mul(sqs[:], acts[:, CQ:CQ + CKV],
                                 acts[:, CQ:CQ + CKV])
            nc.vector.reduce_sum(fk[:], sqs[:], axis=mybir.AxisListType.X)
            nc.vector.tensor_scalar(fk[:], fk[:], 1.0 / CKV, EPS, OP.mult, OP.add)
            nc.vector.reciprocal(fk[:], fk[:])
            nc.scalar.activation(fk[:], fk[:], AF.Sqrt)
            nc.vector.tensor_scalar_mul(acts[:, CQ:CQ + CKV], acts[:, CQ:CQ + CKV], fk[:])

            # k_pe rope (natural [s, 64] layout), cols 2048:2112
            kp0 = CQ + CKV
            kv1 = sbt.tile([S_SH, D_ROPE], F32, tag="kv1")
            kv2 = sbt.tile([S_SH, D_ROPE], F32, tag="kv2")
            nc.vector.tensor_mul(kv1[:], acts[:, kp0:kp0 + 64], cos_s_sb[:])
            nc.vector.tensor_mul(kv2[:, 0:32], acts[:, kp0 + 32:kp0 + 64], sin_sg_sb[:, 0:32])
            nc.vector.tensor_mul(kv2[:, 32:64], acts[:, kp0:kp0 + 32], sin_sg_sb[:, 32:64])
            nc.vector.tensor_add(acts[:, kp0:kp0 + 64], kv1[:], kv2[:])

            for t in range(12, 17):
                w = 128 if t < 16 else 64
                pt_ps = psa.tile([128, 128], F32, tag="tp", name="pt_ps")
                nc.tensor.transpose(pt_ps[:w, 0:128], acts[:, t * 128:t * 128 + w], ident[:])
                nc.scalar.copy(bT[:w, t * 128:(t + 1) * 128], pt_ps[:w, 0:128])
                nc.sync.dma_start(agi2[(t - 12) * 128:(t - 12) * 128 + w, :],
                                  bT[:w, t * 128:(t + 1) * 128])

            nc.gpsimd.collective_compute(
                "AllGather", OP.bypass,
                replica_groups=[list(range(N_CORES))],
                ins=[agi2.opt()], outs=[ago2.opt()],
            )

            # ---- pass 1: q_c columns 0:1536 ----
            # The AG2-dependent stitch DMAs are emitted midway through this
            # loop: by the time the FIFO DMA queues reach them AG2 has long
            # completed, so they never block the pass-1 weight stream.
            ckv = []
            kpe2 = sbg.tile([128, S], BF16, tag="kpe2")

            def emit_ckv_stitch():
                for st in range(2):
                    k_t = sbg.tile([128, CKV // 128, 512], BF16, tag=f"ckv{st}",
                                   name=f"ckv{st}")
                    for r in range(4):
                        core = st * 4 + r
                        _dma_rows_to_3d(nc, k_t[:, :, r * 128:(r + 1) * 128],
                                        ago2[core * 576:core * 576 + CKV, :],
                                        CKV // 128)
                    ckv.append(k_t)
                # k_pe duplicated into both partition halves so each head of
                # a pair gets a base-partition-aligned 64-row stationary
                for core in range(N_CORES):
                    base = core * 576 + CKV
                    nc.sync.dma_start(kpe2[0:64, core * 128:(core + 1) * 128],
                                      ago2[base:base + 64, :])
                    nc.sync.dma_start(kpe2[64:128, core * 128:(core + 1) * 128],
                                      ago2[base:base + 64, :])

            p10 = psa.tile([128, 512], F32, tag="a0", name="p10")
            p11 = psa.tile([128, 512], F32, tag="a1", name="p11")
            p12 = psa.tile([128, 512], F32, tag="a2", name="p12")
            pa_ps = [p10, p11, p12]
            kvw_q = []
            for cc in range(CC_A):
                wt = sbw.tile([128, CQ], BF16, tag="wa1", name="wt")
                nc.sync.dma_start(wt[:, 0:768],
                                  wa.ap()[cc * 128:(cc + 1) * 128, 0:768])
                nc.sync.dma_start(wt[:, 768:CQ],
                                  wa.ap()[cc * 128:(cc + 1) * 128, 768:CQ])
                for j in range(3):
                    nc.tensor.matmul(pa_ps[j][:], xT_sb[:, cc, :],
                                     wt[:, j * 512:(j + 1) * 512],
                                     start=(cc == 0), stop=(cc == CC_A - 1))
                if cc == 20:
                    emit_ckv_stitch()
                elif cc == 26:
                    kvw_q.append(load_group_kvw(0))
                elif cc == 40:
                    kvw_q.append(load_group_kvw(1))
            for j in range(3):
                nc.scalar.copy(acts[:, j * 512:(j + 1) * 512], pa_ps[j][:])

            # q rmsnorm + transpose chunks 0:12 -> agi1
            fq = sbt.tile([S_SH, 1], F32, tag="fq")
            fqp = sbt.tile([S_SH, 4], F32, tag="fqp")
            for j in range(3):
                nc.vector.tensor_mul(sqs[:], acts[:, j * 512:(j + 1) * 512],
                                     acts[:, j * 512:(j + 1) * 512])
                nc.vector.reduce_sum(fqp[:, j:j + 1], sqs[:],
                                     axis=mybir.AxisListType.X)
            nc.vector.tensor_add(fqp[:, 0:1], fqp[:, 0:1], fqp[:, 1:2])
            nc.vector.tensor_add(fq[:], fqp[:, 0:1], fqp[:, 2:3])
            nc.vector.tensor_scalar(fq[:], fq[:], 1.0 / CQ, EPS, OP.mult, OP.add)
            nc.vector.reciprocal(fq[:], fq[:])
            nc.scalar.activation(fq[:], fq[:], AF.Sqrt)
            nc.vector.tensor_scalar_mul(acts[:, 0:CQ], acts[:, 0:CQ], fq[:])

            for t in range(12):
                pt_ps = psa.tile([128, 128], F32, tag="tp", name="pt_ps")
                nc.tensor.transpose(pt_ps[:, 0:128], acts[:, t * 128:(t + 1) * 128], ident[:])
                nc.scalar.copy(bT[:, t * 128:(t + 1) * 128], pt_ps[:, 0:128])
                nc.sync.dma_start(agi1[t * 128:(t + 1) * 128, :], bT[:, t * 128:(t + 1) * 128])

            nc.gpsimd.collective_compute(
                "AllGather", OP.bypass,
                replica_groups=[list(range(N_CORES))],
                ins=[agi1.opt()], outs=[ago1.opt()],
            )

        # kv_b projections for the first 3 groups: pure PE work that only
        # depends on AG2 — it covers AG1's latency. Later groups are produced
        # inside the group loop, 3 groups ahead of their attention.
        emit_kvb(0, *kvw_q[0], ckv)
        kvw_q.append(load_group_kvw(2))
        emit_kvb(1, *kvw_q[1], ckv)
        emit_kvb(2, *kvw_q[2], ckv)

        # phase B/C weight prefetches (DMA queues drain these while AG1 runs)
        wq_pre = [load_group_qw(0), load_group_qw(1)]
        ow_pre = load_ow(0)

        # stitch gathered q_c activations, per 512-wide s-tile
        qct = []
        for st in range(2):
            q_t = sbg.tile([128, CQ // 128, 512], BF16, tag=f"qct{st}",
                           name=f"qct{st}")
            for r in range(4):
                core = st * 4 + r
                _dma_rows_to_3d(nc, q_t[:, :, r * 128:(r + 1) * 128],
                                ago1[core * CQ:(core + 1) * CQ, :], CQ // 128)
            qct.append(q_t)

        # ================= Phase B: q_b projections + attention =============
        with ExitStack() as pb:
            sbh = pb.enter_context(tc.tile_pool(name="sbh", bufs=2))
            sbp = pb.enter_context(tc.tile_pool(name="sbp", bufs=1))
            sbpt = pb.enter_context(tc.tile_pool(name="sbpt", bufs=5))
            sbs = pb.enter_context(tc.tile_pool(name="sbs", bufs=2))
            sbn = pb.enter_context(tc.tile_pool(name="sbn", bufs=3))
            sbo = pb.enter_context(tc.tile_pool(name="sbo", bufs=3))
            ps_o = pb.enter_context(tc.tile_pool(name="ps_o", bufs=2, space="PSUM"))

            pending_norm = []

            def emit_norm_reduce():
                # partition-reduce the denominators on GpSimd (no PE work)
                for idx, (h_idx, qt_, sums_, psum_o_) in enumerate(pending_norm):
                    den = sbn.tile([128, 512], F32, tag="den", name="den")
                    nc.gpsimd.partition_all_reduce(den[:], sums_[:], 128, RED.add)
                    pending_norm[idx] = (h_idx, qt_, den, psum_o_)

            def emit_norm_apply():
                while pending_norm:
                    h_idx, qt_, den, psum_o_ = pending_norm.pop(0)
                    rec = sbn.tile([128, 512], F32, tag="rec", name="rec")
                    nc.vector.reciprocal_approx_fast(rec[:], den[:])
                    nc.vector.tensor_mul(
                        outs_sb[:, h_idx, qt_ * 512:(qt_ + 1) * 512],
                        psum_o_[:], rec[:])

            for g in range(N_GROUPS):
                h0 = g * G_HEADS
                qbnw, qbpw = wq_pre[g] if g < 2 else load_group_qw(g)

                # --- q rope projection first so the DVE rope work is done
                # before the first rope-score matmul needs qTp ---
                qp_raw = sbp.tile([128, S], F32, tag="qp_raw", name="qp_raw")
                p0 = ps_main.tile([128, 512], F32, tag="s", name="p0")
                p1 = ps_main.tile([128, 512], F32, tag="s", name="p1")
                for c in range(CQ // 128):
                    nc.tensor.matmul(p0[:], qbpw[:, c, :], qct[0][:, c, :],
                                     start=(c == 0), stop=(c == CQ // 128 - 1))
                    nc.tensor.matmul(p1[:], qbpw[:, c, :], qct[1][:, c, :],
                                     start=(c == 0), stop=(c == CQ // 128 - 1))
                nc.scalar.copy(qp_raw[:, 0:512], p0[:])
                nc.scalar.copy(qp_raw[:, 512:1024], p1[:])
                emit_norm_reduce()   # prev group's partition reduces (GpSimd)
                # rope on the head-pair tile: rows [0:64]=head h0, [64:128]=h0+1
                qTp = sbh.tile([128, S], BF16, tag="qTp")
                rm = sbp.tile([128, S], F32, tag="ropem")
                rs = sbp.tile([128, S], F32, tag="ropes")
                for hh in range(2):
                    sl = slice(hh * 512, (hh + 1) * 512)
                    nc.vector.tensor_mul(rm[:, sl], qp_raw[:, sl], cos2t_sb[:, sl])
                    for b in range(4):
                        r0 = b * 32
                        r1 = r0 + 32 if b % 2 == 0 else r0 - 32
                        nc.vector.tensor_copy(rs[r0:r0 + 32, sl], qp_raw[r1:r1 + 32, sl])
                    nc.vector.tensor_mul(rs[:, sl], rs[:, sl], sin2tg_sb[:, sl])
                    nc.vector.tensor_add(qTp[:, sl], rm[:, sl], rs[:, sl])

                # --- q nope projections; st-paired so each stationary is
                # loaded once per two 512-streams ---
                qTn = []
                for i in range(G_HEADS):
                    qt_t = sbh.tile([128, S], BF16, tag=f"qTn{i}", name=f"qTn{i}")
                    p0 = ps_main.tile([128, 512], F32, tag="s", name="p0")
                    p1 = ps_main.tile([128, 512], F32, tag="s", name="p1")
                    if g == 0 and i == 0:
                        # first chains after the AllGather: unpaired, so the
                        # st=0 chain starts as soon as half the stitch lands
                        for c in range(CQ // 128):
                            nc.tensor.matmul(p0[:], qbnw[:, c, 0:128],
                                             qct[0][:, c, :],
                                             start=(c == 0), stop=(c == CQ // 128 - 1))
                        for c in range(CQ // 128):
                            nc.tensor.matmul(p1[:], qbnw[:, c, 0:128],
                                             qct[1][:, c, :],
                                             start=(c == 0), stop=(c == CQ // 128 - 1))
                    else:
                        for c in range(CQ // 128):
                            nc.tensor.matmul(p0[:], qbnw[:, c, i * 128:(i + 1) * 128],
                                             qct[0][:, c, :],
                                             start=(c == 0), stop=(c == CQ // 128 - 1))
                            nc.tensor.matmul(p1[:], qbnw[:, c, i * 128:(i + 1) * 128],
                                             qct[1][:, c, :],
                                             start=(c == 0), stop=(c == CQ // 128 - 1))
                    nc.scalar.copy(qt_t[:, 0:512], p0[:])
                    nc.scalar.copy(qt_t[:, 512:1024], p1[:])
                    qTn.append(qt_t)
                    if i == 0:
                        emit_norm_apply()  # prev group's recip+mul (DVE)

                if g + 3 < N_GROUPS:
                    kvw = load_group_kvw(g + 3)
                    emit_kvb(g + 3, *kvw, ckv)
                kT_g, v_g = kv_tiles[g]

                # --- attention: heads interleaved, AV skewed one kc behind,
                # causal windows start at the diagonal ---
                for qt in range(2):
                    kmax = 4 * (qt + 1)
                    sums = [sbs.tile([128, 512], F32, tag=f"sums{i}", name=f"sums{i}")
                            for i in range(G_HEADS)]
                    psum_o = [ps_o.tile([128, 512], F32, tag=f"o{i}", name=f"po{i}")
                              for i in range(G_HEADS)]
                    pt = {}

                    def av_step(kc, last):
                        offp = max(0, (kc - 4 * qt)) * 128
                        for i in range(G_HEADS):
                            nc.tensor.matmul(psum_o[i][:, offp:512],
                                             v_g[:, kc, i * 128:(i + 1) * 128],
                                             pt[(i, kc)][:, offp:512],
                                             start=(kc == 0), stop=last,
                                             skip_group_check=True)

                    for kc in range(kmax):
                        off = max(0, (kc - 4 * qt)) * 128
                        qsl = slice(qt * 512 + off, (qt + 1) * 512)
                        for i in range(G_HEADS):
                            ps = ps_main.tile([128, 512], F32, tag="s", name="ps")
                            nc.tensor.matmul(ps[:, off:512],
                                             kT_g[:, i, kc * 128:(kc + 1) * 128],
                                             qTn[i][:, qsl],
                                             start=True, stop=False)
                            nc.tensor.matmul(ps[:, off:512],
                                             kpe2[i * 64:(i + 1) * 64,
                                                  kc * 128:(kc + 1) * 128],
                                             qTp[i * 64:(i + 1) * 64, qsl],
                                             start=False, stop=True)
                            p = sbpt.tile([128, 512], BF16, tag="pt", name="p")
                            nc.scalar.activation(p[:, off:512], ps[:, off:512],
                                                 AF.Exp, scale=SCALE)
                            if kc >= 4 * qt:
                                nc.vector.tensor_mul(p[:, off:off + 128],
                                                     p[:, off:off + 128], tri_sb[:])
                            if kc == 0:
                                nc.vector.tensor_copy(sums[i][:], p[:])
                            else:
                                nc.vector.tensor_add(sums[i][:, off:512],
                                                     sums[i][:, off:512],
                                                     p[:, off:512])
                            pt[(i, kc)] = p
                        if kc > 0:
                            av_step(kc - 1, last=False)
                    av_step(kmax - 1, last=True)
                    for i in range(G_HEADS):
                        pending_norm.append((h0 + i, qt, sums[i], psum_o[i]))

            emit_norm_reduce()
            emit_norm_apply()

            # ========= Phase C: partial output projection, out^T layout =====
            # out^T[hid, s] = sum_hc ow[hc]^T @ outs[hc]; each ow stationary
            # tile serves two 512-wide s-blocks before the next LDWEIGHTS.
            for nt in range(HID // 256):
                owt = ow_pre if nt == 0 else load_ow(nt)
                for ntl in range(2):
                    pA = ps_main.tile([128, 512], F32, tag="s", name="pA")
                    pB = ps_main.tile([128, 512], F32, tag="s", name="pB")
                    for hc in range(HG):
                        lhs = owt[:, hc, ntl * 128:(ntl + 1) * 128]
                        nc.tensor.matmul(pA[:], lhs, outs_sb[:, hc, 0:512],
                                         start=(hc == 0), stop=(hc == HG - 1))
                        nc.tensor.matmul(pB[:], lhs, outs_sb[:, hc, 512:1024],
                                         start=(hc == 0), stop=(hc == HG - 1))
                    for half, pp in ((0, pA), (1, pB)):
                        osb = sbo.tile([128, 512], BF16, tag="osb", name="osb")
                        nc.scalar.copy(osb[:], pp[:])
                        nc.sync.dma_start(
                            outT.ap()[nt * 256 + ntl * 128:nt * 256 + (ntl + 1) * 128,
                                      half * 512:(half + 1) * 512], osb[:])

    nc.compile()
    return nc


def _host_inputs(hidden_states, position_ids, q_a_weight, q_a_layernorm_weight,
                 q_b_weight, kv_a_weight, kv_a_layernorm_weight, kv_b_weight,
                 o_weight):
    bf16 = ml_dtypes.bfloat16
    x = np.asarray(hidden_states, np.float32).reshape(S, HID)
    pos = np.asarray(position_ids, np.float64).reshape(S)
    q_a_w = np.asarray(q_a_weight, np.float32)
    q_ln = np.asarray(q_a_layernorm_weight, np.float32)
    q_b_w = np.asarray(q_b_weight, np.float32)
    kv_a_w = np.asarray(kv_a_weight, np.float32)
    kv_ln = np.asarray(kv_a_layernorm_weight, np.float32)
    kv_b_w = np.asarray(kv_b_weight, np.float32)
    o_w = np.asarray(o_weight, np.float32)

    wa = np.concatenate([q_a_w, kv_a_w], axis=1).astype(bf16)  # [HID, 2112]
    xT = np.ascontiguousarray(x.T).astype(bf16)                # [HID, S]

    # fold the rms-norm weights into the b-projections
    qb = (q_ln[:, None] * q_b_w).reshape(CQ, H, D_Q)
    kvb = (kv_ln[:, None] * kv_b_w).reshape(CKV, H, D_NOPE + D_V)

    # rope tables
    inv_freq = 1.0 / (10000.0 ** (np.arange(0, D_ROPE, 2, dtype=np.float64) / D_ROPE))
    freqs = pos[:, None] * inv_freq[None, :]                # [S, 32]
    emb = np.concatenate([freqs, freqs], axis=-1)           # [S, 64]
    cos = np.cos(emb).astype(np.float32)
    sin = np.sin(emb).astype(np.float32)
    sin_sg = np.concatenate([-sin[:, :32], sin[:, 32:]], axis=1)  # [S, 64]
    cosT = np.ascontiguousarray(cos.T)                      # [64, S]
    sinT_sg = np.ascontiguousarray(sin_sg.T)                # [64, S]
    cos2t = np.concatenate([cosT, cosT], axis=0)            # [128, S]
    sin2tg = np.concatenate([sinT_sg, sinT_sg], axis=0)     # [128, S]

    # causal triangle for the diagonal 128x128 blocks: key row r valid for
    # query col c iff r <= c
    i = np.arange(128)[:, None]
    j = np.arange(128)[None, :]
    tri = (i <= j).astype(np.float32).astype(bf16)

    in_maps = []
    for c in range(N_CORES):
        hs = slice(c * HG, (c + 1) * HG)
        in_maps.append({
            "xT": np.ascontiguousarray(xT[:, c * S_SH:(c + 1) * S_SH]),
            "wa": wa,
            "qbn": np.ascontiguousarray(
                qb[:, hs, :D_NOPE].reshape(CQ, HG * D_NOPE)).astype(bf16),
            "qbp": np.ascontiguousarray(
                qb[:, hs, D_NOPE:].reshape(CQ, HG * D_ROPE)).astype(bf16),
            "kvbk": np.ascontiguousarray(
                kvb[:, hs, :D_NOPE].reshape(CKV, HG * D_NOPE)).astype(bf16),
            "kvbv": np.ascontiguousarray(
                kvb[:, hs, D_NOPE:].reshape(CKV, HG * D_V)).astype(bf16),
            "ow": np.ascontiguousarray(
                o_w[c * HG * D_V:(c + 1) * HG * D_V, :]).astype(bf16),
            "cos_s": np.ascontiguousarray(cos[c * S_SH:(c + 1) * S_SH, :]),
            "sin_sg": np.ascontiguousarray(sin_sg[c * S_SH:(c + 1) * S_SH, :]),
            "cos2t": cos2t,
            "sin2tg": sin2tg,
            "tri": tri,
        })
    return in_maps


def kernel(**inputs):
    global LAST_EXEC_NS
    trace = bool(inputs.pop("_trace", False))
    in_maps = _host_inputs(**inputs)
    if "nc" not in _CACHE:
        _CACHE["nc"] = _build_nc()
    nc = _CACHE["nc"]
    res = bass_utils.run_bass_kernel_spmd(
        nc, in_maps, core_ids=list(range(N_CORES)), trace=trace)
    LAST_EXEC_NS = res.exec_time_ns
    total = np.zeros((HID, S), np.float64)
    for c in range(N_CORES):
        total += res.results[c]["outT"].astype(np.float64)
    return np.ascontiguousarray(total.T).astype(np.float32).reshape(1, 1, S, HID)


# revision 21
# speedup vs baseline: 1.1384x; 1.0425x over previous
"""DeepseekV3 MLA attention prefill (S=1024, H=128 heads, HID=7168) on 8 TRN2
NeuronCores.

Sharding: tensor-parallel over heads (16 heads/core) for attention and the
output projection (host sums 8 partial outT). The fused a-projection is
COLUMN-sharded (v3): each core computes all 1024 rows for its own 320-column
slab of [q_a | kv_a | k_pe] (cores 0-5 own q columns, 6-7 own kv + k_pe,
64 pad columns on 0-6), so only ~19MB (full x^T + its wa slab) moves per
core instead of the 33MB replicated-wa scheme. The rms denominators cross
cores via an 8KB AllReduce; activations are produced directly in the
transposed [col, s] layout (stationary = wa chunks), so no PE transposes are
needed before the s-split AllGather pair.

Softmax normalization never touches the PE: gpsimd.partition_all_reduce +
fast DVE reciprocal + multiply, deferred one group. Causal windows start at
the diagonal. AV matmuls run one kc-step behind the score matmuls. kv_b
projections for 3 groups run right after the AllGather to cover its tail;
later groups are produced 2 groups ahead inside the loop.
"""
import math
import numpy as np
import ml_dtypes

import concourse.bass as bass
import concourse.mybir as mybir
import concourse.bacc as bacc
import concourse.bass_isa as bass_isa
import concourse.tile as tile
import concourse.bass_utils as bass_utils
from contextlib import ExitStack

F32 = mybir.dt.float32
BF16 = mybir.dt.bfloat16
AF = mybir.ActivationFunctionType
OP = mybir.AluOpType
RED = bass_isa.ReduceOp

N_CORES = 8
S = 1024
HID = 7168
H = 128
HG = H // N_CORES          # 16 heads per core
D_NOPE = 128
D_ROPE = 64
D_Q = D_NOPE + D_ROPE      # 192
D_V = 128
CQ = 1536                  # q lora rank
CKV = 512                  # kv lora rank
CA = CQ + CKV + D_ROPE     # 2112 fused a-proj cols
W_SL = 320                 # per-core a-proj column slab (256 owned + pad)
CC_A = HID // 128          # 56 contraction chunks for a-proj
SCALE = 1.0 / math.sqrt(D_Q)
EPS = 1e-6
G_HEADS = 2                # heads per group
N_GROUPS = HG // G_HEADS   # 8 groups
LAST_EXEC_NS = None

_CACHE = {}


def _dma_rows_to_3d(nc, dst, src_ap, n_chunks, p=128):
    """dst [p, n_chunks, w] <- src rows laid out as (chunk, p)."""
    try:
        nc.sync.dma_start(dst, src_ap.rearrange("(c p) s -> p c s", p=p))
    except Exception:
        for c in range(n_chunks):
            nc.sync.dma_start(dst[:, c, :], src_ap[c * p:(c + 1) * p, :])


def _build_nc():
    nc = bacc.Bacc("TRN2", target_bir_lowering=False, debug=False,
                   num_devices=N_CORES)

    xT = nc.dram_tensor("xT", [HID, S], BF16, kind="ExternalInput")
    wsl = nc.dram_tensor("wsl", [HID, W_SL], BF16, kind="ExternalInput")
    qbn = nc.dram_tensor("qbn", [CQ, HG * D_NOPE], BF16, kind="ExternalInput")
    qbp = nc.dram_tensor("qbp", [CQ, HG * D_ROPE], BF16, kind="ExternalInput")
    kvbk = nc.dram_tensor("kvbk", [CKV, HG * D_NOPE], BF16, kind="ExternalInput")
    kvbv = nc.dram_tensor("kvbv", [CKV, HG * D_V], BF16, kind="ExternalInput")
    ow = nc.dram_tensor("ow", [HG * D_V, HID], BF16, kind="ExternalInput")
    cos2t = nc.dram_tensor("cos2t", [128, S], F32, kind="ExternalInput")
    sin2tg = nc.dram_tensor("sin2tg", [128, S], F32, kind="ExternalInput")
    cosC = nc.dram_tensor("cosC", [D_ROPE, S], F32, kind="ExternalInput")
    sinC = nc.dram_tensor("sinC", [D_ROPE, S], F32, kind="ExternalInput")
    tri = nc.dram_tensor("tri", [128, 128], BF16, kind="ExternalInput")
    mcol = nc.dram_tensor("mcol", [128, 4], BF16, kind="ExternalInput")
    msel = nc.dram_tensor("msel", [128, 2], F32, kind="ExternalInput")
    nscale = nc.dram_tensor("nscale", [2, 1], F32, kind="ExternalInput")
    outT = nc.dram_tensor("outT", [HID, S], BF16, kind="ExternalOutput")

    with tile.TileContext(nc) as tc, ExitStack() as top:
        const = top.enter_context(tc.tile_pool(name="const", bufs=1))
        dram = top.enter_context(tc.tile_pool(name="dram", bufs=1, space="DRAM"))
        outsp = top.enter_context(tc.tile_pool(name="outsp", bufs=1))
        sbwq = top.enter_context(tc.tile_pool(name="sbwq", bufs=2))
        sbow = top.enter_context(tc.tile_pool(name="sbow", bufs=2))
        sbkw = top.enter_context(tc.tile_pool(name="sbkw", bufs=3))
        sbkv = top.enter_context(tc.tile_pool(name="sbkv", bufs=3))
        sbg = top.enter_context(tc.tile_pool(name="sbg", bufs=1))

        # ---- constants in SBUF ----
        tri_sb = const.tile([128, 128], BF16, tag="tri")
        nc.sync.dma_start(tri_sb[:], tri.ap())
        cos2t_sb = const.tile([128, S], F32, tag="cos2t")
        sin2tg_sb = const.tile([128, S], F32, tag="sin2tg")
        nc.sync.dma_start(cos2t_sb[:], cos2t.ap())
        nc.sync.dma_start(sin2tg_sb[:], sin2tg.ap())
        cosC_sb = const.tile([D_ROPE, S], F32, tag="cosC")
        sinC_sb = const.tile([D_ROPE, S], F32, tag="sinC")
        nc.sync.dma_start(cosC_sb[:], cosC.ap())
        nc.sync.dma_start(sinC_sb[:], sinC.ap())
        mcol_sb = const.tile([128, 4], BF16, tag="mcol")
        nc.sync.dma_start(mcol_sb[:], mcol.ap())
        msel_sb = const.tile([128, 2], F32, tag="msel")
        nc.sync.dma_start(msel_sb[:], msel.ap())
        nscale_sb = const.tile([2, 1], F32, tag="nscale")
        nc.sync.dma_start(nscale_sb[:], nscale.ap())

        ari = dram.tile([2, S], F32, tag="ari")
        aro = dram.tile([2, S], F32, tag="aro", addr_space="Shared")
        agiA = dram.tile([W_SL, 512], BF16, tag="agiA")
        agiB = dram.tile([W_SL, 512], BF16, tag="agiB")
        agoA = dram.tile([W_SL * N_CORES, 512], BF16, tag="agoA",
                         addr_space="Shared")
        agoB = dram.tile([W_SL * N_CORES, 512], BF16, tag="agoB",
                         addr_space="Shared")

        # all 16 heads' attention outputs live in SBUF [dv=128, head, s]
        outs_sb = outsp.tile([128, HG, S], BF16, tag="outs")

        def load_group_qw(g):
            h0 = g * G_HEADS
            qbnw = sbwq.tile([128, CQ // 128, G_HEADS * 128], BF16,
                             tag="qbnw", name="qbnw")
            qbpw = sbwq.tile([128, CQ // 128, G_HEADS * 64], BF16,
                             tag="qbpw", name="qbpw")
            _dma_rows_to_3d(nc, qbnw[:],
                            qbn.ap()[:, h0 * 128:(h0 + G_HEADS) * 128], CQ // 128)
            _dma_rows_to_3d(nc, qbpw[:],
                            qbp.ap()[:, h0 * 64:(h0 + G_HEADS) * 64], CQ // 128)
            return qbnw, qbpw

        def load_group_kvw(g):
            h0 = g * G_HEADS
            kvbkw = sbkw.tile([128, CKV // 128, G_HEADS * 128], BF16,
                              tag="kvbkw", name="kvbkw")
            kvbvw = sbkw.tile([128, CKV // 128, G_HEADS * 128], BF16,
                              tag="kvbvw", name="kvbvw")
            _dma_rows_to_3d(nc, kvbkw[:],
                            kvbk.ap()[:, h0 * 128:(h0 + G_HEADS) * 128], CKV // 128)
            _dma_rows_to_3d(nc, kvbvw[:],
                            kvbv.ap()[:, h0 * 128:(h0 + G_HEADS) * 128], CKV // 128)
            return kvbkw, kvbvw

        def load_ow(nt):
            owt = sbow.tile([128, HG, 256], BF16, tag="owt", name="owt")
            _dma_rows_to_3d(nc, owt[:],
                            ow.ap()[:, nt * 256:(nt + 1) * 256], HG)
            return owt

        # kv_b output tiles: 3-slot sliding window (group g+2 is produced at
        # the end of group g's attention)
        kv_tiles = {}

        def emit_kvb_st(g, st, kvbkw, kvbvw, ckv):
            if g not in kv_tiles:
                kT_g = sbkv.tile([128, G_HEADS, S], BF16, tag="kT",
                                 name=f"kT{g}")
                v_g = sbkv.tile([128, 8, G_HEADS * 128], BF16, tag="v",
                                name=f"v{g}")
                kv_tiles[g] = (kT_g, v_g)
            kT_g, v_g = kv_tiles[g]
            for i in range(G_HEADS):
                p = ps_main.tile([128, 512], F32, tag="s", name="pkv")
                for c in range(CKV // 128):
                    nc.tensor.matmul(p[:], kvbkw[:, c, i * 128:(i + 1) * 128],
                                     ckv[st][:, c, :],
                                     start=(c == 0), stop=(c == CKV // 128 - 1))
                nc.scalar.copy(kT_g[:, i, st * 512:(st + 1) * 512], p[:])
            for sc in range(st * 4, st * 4 + 4):
                p = ps_main.tile([128, 512], F32, tag="s", name="pkv")
                nn = G_HEADS * 128
                for c in range(CKV // 128):
                    nc.tensor.matmul(
                        p[:, :nn],
                        ckv[st][:, c, (sc % 4) * 128:(sc % 4 + 1) * 128],
                        kvbvw[:, c, :],
                        start=(c == 0), stop=(c == CKV // 128 - 1))
                nc.scalar.copy(v_g[:, sc, :], p[:, :nn])

        # ================= Phase A: column-sharded a-proj ==================
        with ExitStack() as pa:
            sba = pa.enter_context(tc.tile_pool(name="sba", bufs=1))
            sbst = pa.enter_context(tc.tile_pool(name="sbst", bufs=4))
            psa = pa.enter_context(tc.tile_pool(name="psa", bufs=1, space="PSUM"))

            # psum accumulators: [col, s] layout, held across the whole pass
            pAc = [psa.tile([128, 512], F32, tag=f"pa{j}", name=f"pa{j}")
                   for j in range(4)]            # A-s0, A-s1, B-s0, B-s1
            pCc = [psa.tile([64, 512], F32, tag=f"pc{j}", name=f"pc{j}")
                   for j in range(2)]            # C-s0, C-s1

            for cc in range(CC_A):
                xt_t = sbst.tile([128, S], BF16, tag="xt", name="xt_t")
                nc.sync.dma_start(xt_t[:], xT.ap()[cc * 128:(cc + 1) * 128, :])
                ws_t = sbst.tile([128, W_SL], BF16, tag="ws", name="ws_t")
                nc.sync.dma_start(ws_t[:], wsl.ap()[cc * 128:(cc + 1) * 128, :])
                st_, sp_ = (cc == 0), (cc == CC_A - 1)
                for half in range(2):
                    sl = slice(half * 512, (half + 1) * 512)
                    nc.tensor.matmul(pAc[half][:], ws_t[:, 0:128], xt_t[:, sl],
                                     start=st_, stop=sp_)
                    nc.tensor.matmul(pAc[2 + half][:], ws_t[:, 128:256],
                                     xt_t[:, sl], start=st_, stop=sp_)
                    nc.tensor.matmul(pCc[half][:], ws_t[:, 256:320],
                                     xt_t[:, sl], start=st_, stop=sp_)

            # ---- rms denominators: square, indicator-matmul, AllReduce ----
            sqA = sba.tile([128, S], BF16, tag="sqA")
            sqB = sba.tile([128, S], BF16, tag="sqB")
            for half in range(2):
                sl = slice(half * 512, (half + 1) * 512)
                nc.scalar.activation(sqA[:, sl], pAc[half][:], AF.Square)
                nc.scalar.activation(sqB[:, sl], pAc[2 + half][:], AF.Square)
            psS = [psa.tile([2, 512], F32, tag=f"ss{j}", name=f"ss{j}")
                   for j in range(2)]
            for half in range(2):
                sl = slice(half * 512, (half + 1) * 512)
                nc.tensor.matmul(psS[half][:], mcol_sb[:, 0:2], sqA[:, sl],
                                 start=True, stop=False)
                nc.tensor.matmul(psS[half][:], mcol_sb[:, 2:4], sqB[:, sl],
                                 start=False, stop=True)
            sums_sb = sba.tile([2, S], F32, tag="sums_sb")
            for half in range(2):
                nc.scalar.copy(sums_sb[:, half * 512:(half + 1) * 512],
                               psS[half][:])
            nc.sync.dma_start(ari[:, :], sums_sb[:])
            nc.gpsimd.collective_compute(
                "AllReduce", OP.add,
                replica_groups=[list(range(N_CORES))],
                ins=[ari.opt()], outs=[aro.opt()],
            )

            # prefetches that must land before the stitch consumers run; the
            # FIFO DMA queues reach these after the pass stream, well before
            # the AllGathers complete
            kvw_q = [load_group_kvw(0), load_group_kvw(1)]
            wq_pre = [load_group_qw(0), load_group_qw(1)]
            ow_pre = load_ow(0)

            # ---- k_pe rope in transposed [d, s] layout (chunk C) ----
            # identity tables on cores 0-6 make this a no-op copy there
            kraw = sba.tile([64, S], F32, tag="kraw")
            rrC = sba.tile([64, S], F32, tag="rrC")
            bTC = sba.tile([64, S], BF16, tag="bTC")
            for half in range(2):
                sl = slice(half * 512, (half + 1) * 512)
                nc.vector.tensor_copy(kraw[:, sl], pCc[half][:])
            nc.vector.tensor_copy(rrC[0:32, :], kraw[32:64, :])
            nc.vector.tensor_copy(rrC[32:64, :], kraw[0:32, :])
            nc.vector.tensor_mul(rrC[:], rrC[:], sinC_sb[:])
            nc.vector.tensor_mul(kraw[:], kraw[:], cosC_sb[:])
            nc.vector.tensor_add(bTC[:], kraw[:], rrC[:])

            # ---- factors from the AllReduce, then normalize + cast ----
            sums_rd = sba.tile([2, S], F32, tag="sums_rd")
            nc.sync.dma_start(sums_rd[:], aro[:, :])
            nc.vector.tensor_scalar_mul(sums_rd[:], sums_rd[:], nscale_sb[:])
            nc.vector.tensor_scalar(sums_rd[:], sums_rd[:], 1.0, EPS,
                                    OP.mult, OP.add)
            nc.vector.reciprocal_approx_fast(sums_rd[:], sums_rd[:])
            nc.scalar.activation(sums_rd[:], sums_rd[:], AF.Sqrt)
            fq_bc = sba.tile([128, S], F32, tag="fq_bc")
            fk_bc = sba.tile([128, S], F32, tag="fk_bc")
            fk_row = sba.tile([1, S], F32, tag="fk_row")
            nc.sync.dma_start(fk_row[:], sums_rd[1:2, :])
            nc.gpsimd.partition_broadcast(fq_bc[:], sums_rd[0:1, :])
            nc.gpsimd.partition_broadcast(fk_bc[:], fk_row[0:1, :])
            tB = sba.tile([128, S], F32, tag="tB")
            nc.vector.tensor_sub(fq_bc[:], fq_bc[:], fk_bc[:])
            nc.vector.tensor_scalar_mul(tB[:], fq_bc[:], msel_sb[:, 1:2])
            nc.vector.tensor_add(tB[:], tB[:], fk_bc[:])
            nc.vector.tensor_scalar_mul(fq_bc[:], fq_bc[:], msel_sb[:, 0:1])
            nc.vector.tensor_add(fq_bc[:], fq_bc[:], fk_bc[:])   # now = tA
            bTA = sba.tile([128, S], BF16, tag="bTA")
            bTB = sba.tile([128, S], BF16, tag="bTB")
            for half in range(2):
                sl = slice(half * 512, (half + 1) * 512)
                nc.vector.tensor_mul(bTA[:, sl], pAc[half][:], fq_bc[:, sl])
                nc.vector.tensor_mul(bTB[:, sl], pAc[2 + half][:], tB[:, sl])

            # ---- AllGather inputs (already transposed) ----
            for agi, half in ((agiA, 0), (agiB, 1)):
                sl = slice(half * 512, (half + 1) * 512)
                nc.sync.dma_start(agi[0:128, :], bTA[:, sl])
                nc.sync.dma_start(agi[128:256, :], bTB[:, sl])
                nc.sync.dma_start(agi[256:320, :], bTC[:, sl])
            nc.gpsimd.collective_compute(
                "AllGather", OP.bypass,
                replica_groups=[list(range(N_CORES))],
                ins=[agiA.opt()], outs=[agoA.opt()],
            )
            nc.gpsimd.collective_compute(
                "AllGather", OP.bypass,
                replica_groups=[list(range(N_CORES))],
                ins=[agiB.opt()], outs=[agoB.opt()],
            )

        ps_main = top.enter_context(tc.tile_pool(name="ps_main", bufs=3,
                                                 space="PSUM"))

        # ---- stitch the gathered activations ----
        # global row of col j on core c is c*320 + j; q cols sit on cores
        # 0-5 (2 aligned 128-chunks each), kv on 6-7, k_pe on core 7 rows
        # 256:320 (already rope'd, transposed)
        qct = []
        ckv = []
        kpe2 = sbg.tile([128, S], BF16, tag="kpe2")
        for st, ago in ((0, agoA), (1, agoB)):
            k_t = sbg.tile([128, CKV // 128, 512], BF16, tag=f"ckv{st}",
                           name=f"ckv{st}")
            for c in range(CKV // 128):
                base = (6 + c // 2) * W_SL + (c % 2) * 128
                nc.sync.dma_start(k_t[:, c, :], ago[base:base + 128, :])
            ckv.append(k_t)
            base = 7 * W_SL + 256
            nc.sync.dma_start(kpe2[0:64, st * 512:(st + 1) * 512],
                              ago[base:base + 64, :])
            nc.sync.dma_start(kpe2[64:128, st * 512:(st + 1) * 512],
                              ago[base:base + 64, :])
        for st, ago in ((0, agoA), (1, agoB)):
            q_t = sbg.tile([128, CQ // 128, 512], BF16, tag=f"qct{st}",
                           name=f"qct{st}")
            for c in range(CQ // 128):
                base = (c // 2) * W_SL + (c % 2) * 128
                nc.sync.dma_start(q_t[:, c, :], ago[base:base + 128, :])
            qct.append(q_t)

        # kv_b for groups 0-2 covers the AllGather tail (st0 chains first so
        # they only wait on the first AllGather)
        kvw_q.append(load_group_kvw(2))
        for st in range(2):
            for g in range(3):
                emit_kvb_st(g, st, *kvw_q[g], ckv)

        # ================= Phase B: q_b projections + attention =============
        with ExitStack() as pb:
            sbh = pb.enter_context(tc.tile_pool(name="sbh", bufs=2))
            sbp = pb.enter_context(tc.tile_pool(name="sbp", bufs=1))
            sbpt = pb.enter_context(tc.tile_pool(name="sbpt", bufs=5))
            sbs = pb.enter_context(tc.tile_pool(name="sbs", bufs=2))
            sbn = pb.enter_context(tc.tile_pool(name="sbn", bufs=2))
            sbo = pb.enter_context(tc.tile_pool(name="sbo", bufs=3))
            ps_o = pb.enter_context(tc.tile_pool(name="ps_o", bufs=2, space="PSUM"))

            pending_norm = []

            def emit_norm_reduce():
                for idx, (h_idx, qt_, sums_, psum_o_) in enumerate(pending_norm):
                    den = sbn.tile([128, 512], F32, tag="den", name="den")
                    nc.gpsimd.partition_all_reduce(den[:], sums_[:], 128, RED.add)
                    pending_norm[idx] = (h_idx, qt_, den, psum_o_)

            def emit_norm_apply():
                while pending_norm:
                    h_idx, qt_, den, psum_o_ = pending_norm.pop(0)
                    rec = sbn.tile([128, 512], F32, tag="rec", name="rec")
                    nc.vector.reciprocal_approx_fast(rec[:], den[:])
                    nc.vector.tensor_mul(
                        outs_sb[:, h_idx, qt_ * 512:(qt_ + 1) * 512],
                        psum_o_[:], rec[:])

            for g in range(N_GROUPS):
                h0 = g * G_HEADS
                qbnw, qbpw = wq_pre[g] if g < 2 else load_group_qw(g)

                # --- q rope projection first so the DVE rope work is done
                # before the first rope-score matmul needs qTp ---
                qp_raw = sbp.tile([128, S], F32, tag="qp_raw", name="qp_raw")
                p0 = ps_main.tile([128, 512], F32, tag="s", name="p0")
                p1 = ps_main.tile([128, 512], F32, tag="s", name="p1")
                for c in range(CQ // 128):
                    nc.tensor.matmul(p0[:], qbpw[:, c, :], qct[0][:, c, :],
                                     start=(c == 0), stop=(c == CQ // 128 - 1))
                    nc.tensor.matmul(p1[:], qbpw[:, c, :], qct[1][:, c, :],
                                     start=(c == 0), stop=(c == CQ // 128 - 1))
                nc.scalar.copy(qp_raw[:, 0:512], p0[:])
                nc.scalar.copy(qp_raw[:, 512:1024], p1[:])
                emit_norm_reduce()   # prev group's partition reduces (GpSimd)
                # rope on the head-pair tile: rows [0:64]=head h0, [64:128]=h0+1
                qTp = sbh.tile([128, S], BF16, tag="qTp")
                rm = sbp.tile([128, S], F32, tag="ropem")
                rs = sbp.tile([128, S], F32, tag="ropes")
                for hh in range(2):
                    sl = slice(hh * 512, (hh + 1) * 512)
                    nc.vector.tensor_mul(rm[:, sl], qp_raw[:, sl], cos2t_sb[:, sl])
                    for b in range(4):
                        r0 = b * 32
                        r1 = r0 + 32 if b % 2 == 0 else r0 - 32
                        nc.vector.tensor_copy(rs[r0:r0 + 32, sl], qp_raw[r1:r1 + 32, sl])
                    nc.vector.tensor_mul(rs[:, sl], rs[:, sl], sin2tg_sb[:, sl])
                    nc.vector.tensor_add(qTp[:, sl], rm[:, sl], rs[:, sl])

                # --- q nope projections; st-paired so each stationary is
                # loaded once per two 512-streams ---
                qTn = []
                for i in range(G_HEADS):
                    qt_t = sbh.tile([128, S], BF16, tag=f"qTn{i}", name=f"qTn{i}")
                    p0 = ps_main.tile([128, 512], F32, tag="s", name="p0")
                    p1 = ps_main.tile([128, 512], F32, tag="s", name="p1")
                    if g == 0 and i == 0:
                        # first chains after the AllGather: unpaired, so the
                        # st=0 chain starts as soon as half the stitch lands
                        for c in range(CQ // 128):
                            nc.tensor.matmul(p0[:], qbnw[:, c, 0:128],
                                             qct[0][:, c, :],
                                             start=(c == 0), stop=(c == CQ // 128 - 1))
                        for c in range(CQ // 128):
                            nc.tensor.matmul(p1[:], qbnw[:, c, 0:128],
                                             qct[1][:, c, :],
                                             start=(c == 0), stop=(c == CQ // 128 - 1))
                    else:
                        for c in range(CQ // 128):
                            nc.tensor.matmul(p0[:], qbnw[:, c, i * 128:(i + 1) * 128],
                                             qct[0][:, c, :],
                                             start=(c == 0), stop=(c == CQ // 128 - 1))
                            nc.tensor.matmul(p1[:], qbnw[:, c, i * 128:(i + 1) * 128],
                                             qct[1][:, c, :],
                                             start=(c == 0), stop=(c == CQ // 128 - 1))
                    nc.scalar.copy(qt_t[:, 0:512], p0[:])
                    nc.scalar.copy(qt_t[:, 512:1024], p1[:])
                    qTn.append(qt_t)
                    if i == 0:
                        emit_norm_apply()  # prev group's recip+mul (DVE)

                kT_g, v_g = kv_tiles[g]

                # --- attention: heads interleaved, AV skewed one kc behind,
                # causal windows start at the diagonal ---
                for qt in range(2):
                    kmax = 4 * (qt + 1)
                    sums = [sbs.tile([128, 512], F32, tag=f"sums{i}", name=f"sums{i}")
                            for i in range(G_HEADS)]
                    psum_o = [ps_o.tile([128, 512], F32, tag=f"o{i}", name=f"po{i}")
                              for i in range(G_HEADS)]
                    pt = {}

                    def av_step(kc, last):
                        offp = max(0, (kc - 4 * qt)) * 128
                        for i in range(G_HEADS):
                            nc.tensor.matmul(psum_o[i][:, offp:512],
                                             v_g[:, kc, i * 128:(i + 1) * 128],
                                             pt[(i, kc)][:, offp:512],
                                             start=(kc == 0), stop=last,
                                             skip_group_check=True)

                    for kc in range(kmax):
                        off = max(0, (kc - 4 * qt)) * 128
                        qsl = slice(qt * 512 + off, (qt + 1) * 512)
                        for i in range(G_HEADS):
                            ps = ps_main.tile([128, 512], F32, tag="s", name="ps")
                            nc.tensor.matmul(ps[:, off:512],
                                             kT_g[:, i, kc * 128:(kc + 1) * 128],
                                             qTn[i][:, qsl],
                                             start=True, stop=False)
                            nc.tensor.matmul(ps[:, off:512],
                                             kpe2[i * 64:(i + 1) * 64,
                                                  kc * 128:(kc + 1) * 128],
                                             qTp[i * 64:(i + 1) * 64, qsl],
                                             start=False, stop=True)
                            p = sbpt.tile([128, 512], BF16, tag="pt", name="p")
                            nc.scalar.activation(p[:, off:512], ps[:, off:512],
                                                 AF.Exp, scale=SCALE)
                            if kc >= 4 * qt:
                                nc.vector.tensor_mul(p[:, off:off + 128],
                                                     p[:, off:off + 128], tri_sb[:])
                            if kc == 0:
                                nc.vector.tensor_copy(sums[i][:], p[:])
                            else:
                                nc.vector.tensor_add(sums[i][:, off:512],
                                                     sums[i][:, off:512],
                                                     p[:, off:512])
                            pt[(i, kc)] = p
                        if kc > 0:
                            av_step(kc - 1, last=False)
                    av_step(kmax - 1, last=True)
                    for i in range(G_HEADS):
                        pending_norm.append((h0 + i, qt, sums[i], psum_o[i]))

                # produce kv for group g+2 (slides the 3-buffer window)
                if g + 2 < N_GROUPS and g + 2 >= 3:
                    kvw = load_group_kvw(g + 2)
                    for st in range(2):
                        emit_kvb_st(g + 2, st, *kvw, ckv)

            emit_norm_reduce()
            emit_norm_apply()

            # ========= Phase C: partial output projection, out^T layout =====
            for nt in range(HID // 256):
                owt = ow_pre if nt == 0 else load_ow(nt)
                for ntl in range(2):
                    pA = ps_main.tile([128, 512], F32, tag="s", name="pA")
                    pB = ps_main.tile([128, 512], F32, tag="s", name="pB")
                    for hc in range(HG):
                        lhs = owt[:, hc, ntl * 128:(ntl + 1) * 128]
                        nc.tensor.matmul(pA[:], lhs, outs_sb[:, hc, 0:512],
                                         start=(hc == 0), stop=(hc == HG - 1))
                        nc.tensor.matmul(pB[:], lhs, outs_sb[:, hc, 512:1024],
                                         start=(hc == 0), stop=(hc == HG - 1))
                    for half, pp in ((0, pA), (1, pB)):
                        osb = sbo.tile([128, 512], BF16, tag="osb", name="osb")
                        nc.scalar.copy(osb[:], pp[:])
                        nc.sync.dma_start(
                            outT.ap()[nt * 256 + ntl * 128:nt * 256 + (ntl + 1) * 128,
                                      half * 512:(half + 1) * 512], osb[:])

    nc.compile()
    return nc


def _host_inputs(hidden_states, position_ids, q_a_weight, q_a_layernorm_weight,
                 q_b_weight, kv_a_weight, kv_a_layernorm_weight, kv_b_weight,
                 o_weight):
    bf16 = ml_dtypes.bfloat16
    x = np.asarray(hidden_states, np.float32).reshape(S, HID)
    pos = np.asarray(position_ids, np.float64).reshape(S)
    q_a_w = np.asarray(q_a_weight, np.float32)
    q_ln = np.asarray(q_a_layernorm_weight, np.float32)
    q_b_w = np.asarray(q_b_weight, np.float32)
    kv_a_w = np.asarray(kv_a_weight, np.float32)
    kv_ln = np.asarray(kv_a_layernorm_weight, np.float32)
    kv_b_w = np.asarray(kv_b_weight, np.float32)
    o_w = np.asarray(o_weight, np.float32)

    wa = np.concatenate([q_a_w, kv_a_w], axis=1)               # [HID, 2112]
    xT = np.ascontiguousarray(x.T).astype(bf16)                # [HID, S]

    # per-core 320-wide wa column slabs (cores 0-6: 256 owned + 64 pad)
    slabs = np.zeros((N_CORES, HID, W_SL), np.float32)
    for c in range(7):
        slabs[c, :, 0:256] = wa[:, c * 256:(c + 1) * 256]
    slabs[7] = wa[:, 1792:2112]

    # fold the rms-norm weights into the b-projections
    qb = (q_ln[:, None] * q_b_w).reshape(CQ, H, D_Q)
    kvb = (kv_ln[:, None] * kv_b_w).reshape(CKV, H, D_NOPE + D_V)

    # rope tables
    inv_freq = 1.0 / (10000.0 ** (np.arange(0, D_ROPE, 2, dtype=np.float64) / D_ROPE))
    freqs = pos[:, None] * inv_freq[None, :]                # [S, 32]
    emb = np.concatenate([freqs, freqs], axis=-1)           # [S, 64]
    cos = np.cos(emb).astype(np.float32)
    sin = np.sin(emb).astype(np.float32)
    sin_sg = np.concatenate([-sin[:, :32], sin[:, 32:]], axis=1)  # [S, 64]
    cosT = np.ascontiguousarray(cos.T)                      # [64, S]
    sinT_sg = np.ascontiguousarray(sin_sg.T)                # [64, S]
    cos2t = np.concatenate([cosT, cosT], axis=0)            # [128, S]
    sin2tg = np.concatenate([sinT_sg, sinT_sg], axis=0)     # [128, S]
    cos_id = np.ones((D_ROPE, S), np.float32)
    sin_id = np.zeros((D_ROPE, S), np.float32)

    # causal triangle for the diagonal 128x128 blocks: key row r valid for
    # query col c iff r <= c
    i = np.arange(128)[:, None]
    j = np.arange(128)[None, :]
    tri = (i <= j).astype(np.float32).astype(bf16)

    # rms indicator stationaries + factor-select masks, per core
    # chunks A (local cols 0:128) and B (128:256): q on cores 0-5, kv on 6-7
    nscale = np.array([[1.0 / CQ], [1.0 / CKV]], np.float32)

    in_maps = []
    for c in range(N_CORES):
        hs = slice(c * HG, (c + 1) * HG)
        is_q = 1.0 if c < 6 else 0.0
        mcol = np.zeros((128, 4), np.float32)
        mcol[:, 0] = is_q          # chunk A contributes to q-sum
        mcol[:, 1] = 1.0 - is_q    # chunk A contributes to kv-sum
        mcol[:, 2] = is_q          # chunk B
        mcol[:, 3] = 1.0 - is_q
        msel = np.zeros((128, 2), np.float32)
        msel[:, 0] = is_q          # factor select for chunk A (1->fq, 0->fk)
        msel[:, 1] = is_q          # chunk B
        in_maps.append({
            "xT": xT,
            "wsl": slabs[c].astype(bf16),
            "qbn": np.ascontiguousarray(
                qb[:, hs, :D_NOPE].reshape(CQ, HG * D_NOPE)).astype(bf16),
            "qbp": np.ascontiguousarray(
                qb[:, hs, D_NOPE:].reshape(CQ, HG * D_ROPE)).astype(bf16),
            "kvbk": np.ascontiguousarray(
                kvb[:, hs, :D_NOPE].reshape(CKV, HG * D_NOPE)).astype(bf16),
            "kvbv": np.ascontiguousarray(
                kvb[:, hs, D_NOPE:].reshape(CKV, HG * D_V)).astype(bf16),
            "ow": np.ascontiguousarray(
                o_w[c * HG * D_V:(c + 1) * HG * D_V, :]).astype(bf16),
            "cos2t": cos2t,
            "sin2tg": sin2tg,
            "cosC": cosT if c == 7 else cos_id,
            "sinC": sinT_sg if c == 7 else sin_id,
            "tri": tri,
            "mcol": mcol.astype(bf16),
            "msel": msel,
            "nscale": nscale,
        })
    return in_maps


def kernel(**inputs):
    global LAST_EXEC_NS
    trace = bool(inputs.pop("_trace", False))
    in_maps = _host_inputs(**inputs)
    if "nc" not in _CACHE:
        _CACHE["nc"] = _build_nc()
    nc = _CACHE["nc"]
    res = bass_utils.run_bass_kernel_spmd(
        nc, in_maps, core_ids=list(range(N_CORES)), trace=trace)
    LAST_EXEC_NS = res.exec_time_ns
    total = np.zeros((HID, S), np.float64)
    for c in range(N_CORES):
        total += res.results[c]["outT"].astype(np.float64)
    return np.ascontiguousarray(total.T).astype(np.float32).reshape(1, 1, S, HID)


# revision 26
# speedup vs baseline: 1.1630x; 1.0216x over previous
"""DeepseekV3 MLA attention prefill (S=1024, H=128 heads, HID=7168) on 8 TRN2
NeuronCores.

Sharding: tensor-parallel over heads (16 heads/core) for attention and the
output projection (host sums 8 partial outT). The fused a-projection is
COLUMN-sharded (v3): each core computes all 1024 rows for its own 320-column
slab of [q_a | kv_a | k_pe] (cores 0-5 own q columns, 6-7 own kv + k_pe,
64 pad columns on 0-6), so only ~19MB (full x^T + its wa slab) moves per
core instead of the 33MB replicated-wa scheme. The rms denominators cross
cores via an 8KB AllReduce; activations are produced directly in the
transposed [col, s] layout (stationary = wa chunks), so no PE transposes are
needed before the s-split AllGather pair.

Softmax normalization never touches the PE: gpsimd.partition_all_reduce +
fast DVE reciprocal + multiply, deferred one group. Causal windows start at
the diagonal. AV matmuls run one kc-step behind the score matmuls. kv_b
projections for 3 groups run right after the AllGather to cover its tail;
later groups are produced 2 groups ahead inside the loop.
"""
import math
import numpy as np
import ml_dtypes

import concourse.bass as bass
import concourse.mybir as mybir
import concourse.bacc as bacc
import concourse.bass_isa as bass_isa
import concourse.tile as tile
import concourse.bass_utils as bass_utils
from contextlib import ExitStack

F32 = mybir.dt.float32
BF16 = mybir.dt.bfloat16
AF = mybir.ActivationFunctionType
OP = mybir.AluOpType
RED = bass_isa.ReduceOp

N_CORES = 8
S = 1024
HID = 7168
H = 128
HG = H // N_CORES          # 16 heads per core
D_NOPE = 128
D_ROPE = 64
D_Q = D_NOPE + D_ROPE      # 192
D_V = 128
CQ = 1536                  # q lora rank
CKV = 512                  # kv lora rank
CA = CQ + CKV + D_ROPE     # 2112 fused a-proj cols
W_SL = 320                 # per-core a-proj column slab (256 owned + pad)
CC_A = HID // 128          # 56 contraction chunks for a-proj
SCALE = 1.0 / math.sqrt(D_Q)
EPS = 1e-6
G_HEADS = 2                # heads per group
N_GROUPS = HG // G_HEADS   # 8 groups
LAST_EXEC_NS = None

_CACHE = {}


def _dma_rows_to_3d(nc, dst, src_ap, n_chunks, p=128):
    """dst [p, n_chunks, w] <- src rows laid out as (chunk, p)."""
    try:
        nc.sync.dma_start(dst, src_ap.rearrange("(c p) s -> p c s", p=p))
    except Exception:
        for c in range(n_chunks):
            nc.sync.dma_start(dst[:, c, :], src_ap[c * p:(c + 1) * p, :])


def _build_nc():
    nc = bacc.Bacc("TRN2", target_bir_lowering=False, debug=False,
                   num_devices=N_CORES)

    xT = nc.dram_tensor("xT", [HID, S], BF16, kind="ExternalInput")
    wsl = nc.dram_tensor("wsl", [HID, W_SL], BF16, kind="ExternalInput")
    qbn = nc.dram_tensor("qbn", [CQ, HG * D_NOPE], BF16, kind="ExternalInput")
    qbp = nc.dram_tensor("qbp", [CQ, HG * D_ROPE], BF16, kind="ExternalInput")
    kvbk = nc.dram_tensor("kvbk", [CKV, HG * D_NOPE], BF16, kind="ExternalInput")
    kvbv = nc.dram_tensor("kvbv", [CKV, HG * D_V], BF16, kind="ExternalInput")
    ow = nc.dram_tensor("ow", [HG * D_V, HID], BF16, kind="ExternalInput")
    cos2t = nc.dram_tensor("cos2t", [128, S], F32, kind="ExternalInput")
    sin2tg = nc.dram_tensor("sin2tg", [128, S], F32, kind="ExternalInput")
    cosC = nc.dram_tensor("cosC", [D_ROPE, S], F32, kind="ExternalInput")
    sinC = nc.dram_tensor("sinC", [D_ROPE, S], F32, kind="ExternalInput")
    tri = nc.dram_tensor("tri", [128, 128], BF16, kind="ExternalInput")
    coremask = nc.dram_tensor("coremask", [N_CORES, 2], F32,
                               kind="ExternalInput")
    outT = nc.dram_tensor("outT", [HID, S], BF16, kind="ExternalOutput")

    with tile.TileContext(nc) as tc, ExitStack() as top:
        const = top.enter_context(tc.tile_pool(name="const", bufs=1))
        dram = top.enter_context(tc.tile_pool(name="dram", bufs=1, space="DRAM"))
        outsp = top.enter_context(tc.tile_pool(name="outsp", bufs=1))
        sbwq = top.enter_context(tc.tile_pool(name="sbwq", bufs=2))
        sbow = top.enter_context(tc.tile_pool(name="sbow", bufs=2))
        sbkw = top.enter_context(tc.tile_pool(name="sbkw", bufs=3))
        sbkv = top.enter_context(tc.tile_pool(name="sbkv", bufs=3))
        sbg = top.enter_context(tc.tile_pool(name="sbg", bufs=1))
        sbf = top.enter_context(tc.tile_pool(name="sbf", bufs=1))

        # ---- constants in SBUF ----
        tri_sb = const.tile([128, 128], BF16, tag="tri")
        nc.sync.dma_start(tri_sb[:], tri.ap())
        cos2t_sb = const.tile([128, S], F32, tag="cos2t")
        sin2tg_sb = const.tile([128, S], F32, tag="sin2tg")
        nc.sync.dma_start(cos2t_sb[:], cos2t.ap())
        nc.sync.dma_start(sin2tg_sb[:], sin2tg.ap())
        cosC_sb = const.tile([D_ROPE, S], F32, tag="cosC")
        sinC_sb = const.tile([D_ROPE, S], F32, tag="sinC")
        nc.sync.dma_start(cosC_sb[:], cosC.ap())
        nc.sync.dma_start(sinC_sb[:], sinC.ap())
        coremask_sb = const.tile([N_CORES, 2], F32, tag="coremask")
        nc.sync.dma_start(coremask_sb[:], coremask.ap())

        agS = dram.tile([1, S], F32, tag="agS")
        agoS = dram.tile([N_CORES, S], F32, tag="agoS", addr_space="Shared")
        agiA = dram.tile([W_SL, 512], BF16, tag="agiA")
        agiB = dram.tile([W_SL, 512], BF16, tag="agiB")
        agoA = dram.tile([W_SL * N_CORES, 512], BF16, tag="agoA",
                         addr_space="Shared")
        agoB = dram.tile([W_SL * N_CORES, 512], BF16, tag="agoB",
                         addr_space="Shared")

        # all 16 heads' attention outputs live in SBUF [dv=128, head, s]
        outs_sb = outsp.tile([128, HG, S], BF16, tag="outs")

        def load_group_qw(g):
            h0 = g * G_HEADS
            qbnw = sbwq.tile([128, CQ // 128, G_HEADS * 128], BF16,
                             tag="qbnw", name="qbnw")
            qbpw = sbwq.tile([128, CQ // 128, G_HEADS * 64], BF16,
                             tag="qbpw", name="qbpw")
            _dma_rows_to_3d(nc, qbnw[:],
                            qbn.ap()[:, h0 * 128:(h0 + G_HEADS) * 128], CQ // 128)
            _dma_rows_to_3d(nc, qbpw[:],
                            qbp.ap()[:, h0 * 64:(h0 + G_HEADS) * 64], CQ // 128)
            return qbnw, qbpw

        def load_group_kvw(g):
            h0 = g * G_HEADS
            kvbkw = sbkw.tile([128, CKV // 128, G_HEADS * 128], BF16,
                              tag="kvbkw", name="kvbkw")
            kvbvw = sbkw.tile([128, CKV // 128, G_HEADS * 128], BF16,
                              tag="kvbvw", name="kvbvw")
            _dma_rows_to_3d(nc, kvbkw[:],
                            kvbk.ap()[:, h0 * 128:(h0 + G_HEADS) * 128], CKV // 128)
            _dma_rows_to_3d(nc, kvbvw[:],
                            kvbv.ap()[:, h0 * 128:(h0 + G_HEADS) * 128], CKV // 128)
            return kvbkw, kvbvw

        def load_ow(nt):
            owt = sbow.tile([128, HG, 256], BF16, tag="owt", name="owt")
            _dma_rows_to_3d(nc, owt[:],
                            ow.ap()[:, nt * 256:(nt + 1) * 256], HG)
            return owt

        # kv_b output tiles: 3-slot sliding window (group g+2 is produced at
        # the end of group g's attention)
        kv_tiles = {}

        def emit_kvb_st(g, st, kvbkw, kvbvw, ckv):
            if g not in kv_tiles:
                kT_g = sbkv.tile([128, G_HEADS, S], BF16, tag="kT",
                                 name=f"kT{g}")
                v_g = sbkv.tile([128, 8, G_HEADS * 128], BF16, tag="v",
                                name=f"v{g}")
                kv_tiles[g] = (kT_g, v_g)
            kT_g, v_g = kv_tiles[g]
            for i in range(G_HEADS):
                p = ps_main.tile([128, 512], F32, tag="s", name="pkv")
                for c in range(CKV // 128):
                    nc.tensor.matmul(p[:], kvbkw[:, c, i * 128:(i + 1) * 128],
                                     ckv[st][:, c, :],
                                     start=(c == 0), stop=(c == CKV // 128 - 1))
                nc.scalar.copy(kT_g[:, i, st * 512:(st + 1) * 512], p[:])
            for sc in range(st * 4, st * 4 + 4):
                p = ps_main.tile([128, 512], F32, tag="s", name="pkv")
                nn = G_HEADS * 128
                for c in range(CKV // 128):
                    nc.tensor.matmul(
                        p[:, :nn],
                        ckv[st][:, c, (sc % 4) * 128:(sc % 4 + 1) * 128],
                        kvbvw[:, c, :],
                        start=(c == 0), stop=(c == CKV // 128 - 1))
                nc.scalar.copy(v_g[:, sc, :], p[:, :nn])

        # ================= Phase A: column-sharded a-proj ==================
        with ExitStack() as pa:
            sba = pa.enter_context(tc.tile_pool(name="sba", bufs=1))
            sbst = pa.enter_context(tc.tile_pool(name="sbst", bufs=4))
            psa = pa.enter_context(tc.tile_pool(name="psa", bufs=1, space="PSUM"))

            # psum accumulators: [col, s] layout, held across the whole pass
            pAc = [psa.tile([128, 512], F32, tag=f"pa{j}", name=f"pa{j}")
                   for j in range(4)]            # A-s0, A-s1, B-s0, B-s1
            pCc = [psa.tile([64, 512], F32, tag=f"pc{j}", name=f"pc{j}")
                   for j in range(2)]            # C-s0, C-s1

            for cc in range(CC_A):
                xt_t = sbst.tile([128, S], BF16, tag="xt", name="xt_t")
                nc.sync.dma_start(xt_t[:], xT.ap()[cc * 128:(cc + 1) * 128, :])
                ws_t = sbst.tile([128, W_SL], BF16, tag="ws", name="ws_t")
                nc.sync.dma_start(ws_t[:], wsl.ap()[cc * 128:(cc + 1) * 128, :])
                st_, sp_ = (cc == 0), (cc == CC_A - 1)
                for half in range(2):
                    sl = slice(half * 512, (half + 1) * 512)
                    nc.tensor.matmul(pAc[half][:], ws_t[:, 0:128], xt_t[:, sl],
                                     start=st_, stop=sp_)
                    nc.tensor.matmul(pAc[2 + half][:], ws_t[:, 128:256],
                                     xt_t[:, sl], start=st_, stop=sp_)
                    nc.tensor.matmul(pCc[half][:], ws_t[:, 256:320],
                                     xt_t[:, sl], start=st_, stop=sp_)

            # ---- rms denominators: each core's A/B columns are homogeneous
            # (all-q or all-kv), so a plain ones-column reduce gives this
            # core's own sums; routing to q vs kv happens receiver-side ----
            sqA = sba.tile([128, S], BF16, tag="sqA")
            sqB = sba.tile([128, S], BF16, tag="sqB")
            for half in range(2):
                sl = slice(half * 512, (half + 1) * 512)
                nc.scalar.activation(sqA[:, sl], pAc[half][:], AF.Square)
                nc.scalar.activation(sqB[:, sl], pAc[2 + half][:], AF.Square)
            psS = [psa.tile([1, 512], F32, tag=f"ss{j}", name=f"ss{j}")
                   for j in range(2)]
            for half in range(2):
                sl = slice(half * 512, (half + 1) * 512)
                nc.tensor.matmul(psS[half][:], tri_sb[:, 127:128], sqA[:, sl],
                                 start=True, stop=False)
                nc.tensor.matmul(psS[half][:], tri_sb[:, 127:128], sqB[:, sl],
                                 start=False, stop=True)
            sums_sb = sba.tile([1, S], F32, tag="sums_sb")
            for half in range(2):
                nc.scalar.copy(sums_sb[0:1, half * 512:(half + 1) * 512],
                               psS[half][:])
            nc.sync.dma_start(agS[:, :], sums_sb[:])
            nc.gpsimd.collective_compute(
                "AllGather", OP.bypass,
                replica_groups=[list(range(N_CORES))],
                ins=[agS.opt()], outs=[agoS.opt()],
            )

            # ---- bf16 casts of the (unnormalized) activations ----
            bTA = sba.tile([128, S], BF16, tag="bTA")
            bTB = sba.tile([128, S], BF16, tag="bTB")
            for half in range(2):
                sl = slice(half * 512, (half + 1) * 512)
                nc.vector.tensor_copy(bTA[:, sl], pAc[half][:])
                nc.vector.tensor_copy(bTB[:, sl], pAc[2 + half][:])

            # ---- k_pe rope in transposed [d, s] layout (chunk C) ----
            # identity tables on cores 0-6 make this a no-op copy there
            kraw = sba.tile([64, S], F32, tag="kraw")
            rrC = sba.tile([64, S], F32, tag="rrC")
            bTC = sba.tile([64, S], BF16, tag="bTC")
            for half in range(2):
                sl = slice(half * 512, (half + 1) * 512)
                nc.vector.tensor_copy(kraw[:, sl], pCc[half][:])
            nc.vector.tensor_copy(rrC[0:32, :], kraw[32:64, :])
            nc.vector.tensor_copy(rrC[32:64, :], kraw[0:32, :])
            nc.vector.tensor_mul(rrC[:], rrC[:], sinC_sb[:])
            nc.vector.tensor_mul(kraw[:], kraw[:], cosC_sb[:])
            nc.vector.tensor_add(bTC[:], kraw[:], rrC[:])

            # ---- AllGather inputs (already transposed, raw) ----
            for agi, half in ((agiA, 0), (agiB, 1)):
                sl = slice(half * 512, (half + 1) * 512)
                nc.sync.dma_start(agi[0:128, :], bTA[:, sl])
                nc.sync.dma_start(agi[128:256, :], bTB[:, sl])
                nc.sync.dma_start(agi[256:320, :], bTC[:, sl])
            nc.gpsimd.collective_compute(
                "AllGather", OP.bypass,
                replica_groups=[list(range(N_CORES))],
                ins=[agiA.opt()], outs=[agoA.opt()],
            )
            nc.gpsimd.collective_compute(
                "AllGather", OP.bypass,
                replica_groups=[list(range(N_CORES))],
                ins=[agiB.opt()], outs=[agoB.opt()],
            )

            # prefetches (FIFO DMA queues reach these behind the agi inputs,
            # well before the AllGathers complete)
            kvw_q = [load_group_kvw(0), load_group_kvw(1)]
            wq_pre = [load_group_qw(0), load_group_qw(1)]
            ow_pre = load_ow(0)

        ps_main = top.enter_context(tc.tile_pool(name="ps_main", bufs=3,
                                                 space="PSUM"))

        # ---- rms factors from the gathered per-core sums (overlaps the big
        # AllGathers): mask by core type, partition-reduce, rsqrt, broadcast.
        # fq_bc/fk_bc persist: the q factor is folded into every qTn/qp psum
        # copy; the kv factor is applied once to the stitched ckv tiles.
        fq_bc = sbf.tile([128, S], F32, tag="fq_bc")
        fk_bc = sbf.tile([128, S], F32, tag="fk_bc")
        with ExitStack() as pf:
            sbr = pf.enter_context(tc.tile_pool(name="sbr", bufs=1))
            sums8 = sbr.tile([N_CORES, S], F32, tag="sums8")
            nc.sync.dma_start(sums8[:], agoS[:, :])
            tq8 = sbr.tile([N_CORES, S], F32, tag="tq8")
            tk8 = sbr.tile([N_CORES, S], F32, tag="tk8")
            nc.vector.tensor_scalar_mul(tq8[:], sums8[:], coremask_sb[:, 0:1])
            nc.vector.tensor_scalar_mul(tk8[:], sums8[:], coremask_sb[:, 1:2])
            dq8 = sbr.tile([N_CORES, S], F32, tag="dq8")
            dk8 = sbr.tile([N_CORES, S], F32, tag="dk8")
            nc.gpsimd.partition_all_reduce(dq8[:], tq8[:], N_CORES, RED.add)
            nc.gpsimd.partition_all_reduce(dk8[:], tk8[:], N_CORES, RED.add)
            for d8, n in ((dq8, CQ), (dk8, CKV)):
                nc.vector.tensor_scalar(d8[0:1, :], d8[0:1, :], 1.0 / n, EPS,
                                        OP.mult, OP.add)
                nc.vector.reciprocal_approx_fast(d8[0:1, :], d8[0:1, :])
                nc.scalar.activation(d8[0:1, :], d8[0:1, :], AF.Sqrt)
            nc.gpsimd.partition_broadcast(fq_bc[:], dq8[0:1, :])
            nc.gpsimd.partition_broadcast(fk_bc[:], dk8[0:1, :])

        # ---- stitch the gathered activations ----
        # global row of col j on core c is c*320 + j; q cols sit on cores
        # 0-5 (2 aligned 128-chunks each), kv on 6-7, k_pe on core 7 rows
        # 256:320 (already rope'd, transposed)
        qct = []
        ckv = []
        kpe2 = sbg.tile([128, S], BF16, tag="kpe2")
        for st, ago in ((0, agoA), (1, agoB)):
            k_t = sbg.tile([128, CKV // 128, 512], BF16, tag=f"ckv{st}",
                           name=f"ckv{st}")
            for c in range(CKV // 128):
                base = (6 + c // 2) * W_SL + (c % 2) * 128
                nc.sync.dma_start(k_t[:, c, :], ago[base:base + 128, :])
            # fold the kv rms factor into the stitched tiles once
            for c in range(CKV // 128):
                nc.vector.tensor_mul(k_t[:, c, :], k_t[:, c, :],
                                     fk_bc[:, st * 512:(st + 1) * 512])
            ckv.append(k_t)
            base = 7 * W_SL + 256
            nc.sync.dma_start(kpe2[0:64, st * 512:(st + 1) * 512],
                              ago[base:base + 64, :])
            nc.sync.dma_start(kpe2[64:128, st * 512:(st + 1) * 512],
                              ago[base:base + 64, :])
        for st, ago in ((0, agoA), (1, agoB)):
            q_t = sbg.tile([128, CQ // 128, 512], BF16, tag=f"qct{st}",
                           name=f"qct{st}")
            for c in range(CQ // 128):
                base = (c // 2) * W_SL + (c % 2) * 128
                nc.sync.dma_start(q_t[:, c, :], ago[base:base + 128, :])
            qct.append(q_t)

        # kv_b for groups 0-2 covers the AllGather tail (st0 chains first so
        # they only wait on the first AllGather)
        kvw_q.append(load_group_kvw(2))
        for st in range(2):
            for g in range(3):
                emit_kvb_st(g, st, *kvw_q[g], ckv)

        # ================= Phase B: q_b projections + attention =============
        with ExitStack() as pb:
            sbh = pb.enter_context(tc.tile_pool(name="sbh", bufs=2))
            sbp = pb.enter_context(tc.tile_pool(name="sbp", bufs=1))
            sbpt = pb.enter_context(tc.tile_pool(name="sbpt", bufs=5))
            sbs = pb.enter_context(tc.tile_pool(name="sbs", bufs=2))
            sbn = pb.enter_context(tc.tile_pool(name="sbn", bufs=2))
            sbo = pb.enter_context(tc.tile_pool(name="sbo", bufs=3))
            ps_o = pb.enter_context(tc.tile_pool(name="ps_o", bufs=2, space="PSUM"))

            pending_norm = []

            def emit_norm_reduce():
                for idx, (h_idx, qt_, sums_, psum_o_) in enumerate(pending_norm):
                    den = sbn.tile([128, 512], F32, tag="den", name="den")
                    nc.gpsimd.partition_all_reduce(den[:], sums_[:], 128, RED.add)
                    pending_norm[idx] = (h_idx, qt_, den, psum_o_)

            def emit_norm_apply():
                while pending_norm:
                    h_idx, qt_, den, psum_o_ = pending_norm.pop(0)
                    rec = sbn.tile([128, 512], F32, tag="rec", name="rec")
                    nc.vector.reciprocal_approx_fast(rec[:], den[:])
                    nc.vector.tensor_mul(
                        outs_sb[:, h_idx, qt_ * 512:(qt_ + 1) * 512],
                        psum_o_[:], rec[:])

            for g in range(N_GROUPS):
                h0 = g * G_HEADS
                qbnw, qbpw = wq_pre[g] if g < 2 else load_group_qw(g)

                # --- q rope projection first so the DVE rope work is done
                # before the first rope-score matmul needs qTp ---
                qp_raw = sbp.tile([128, S], F32, tag="qp_raw", name="qp_raw")
                p0 = ps_main.tile([128, 512], F32, tag="s", name="p0")
                p1 = ps_main.tile([128, 512], F32, tag="s", name="p1")
                for c in range(CQ // 128):
                    nc.tensor.matmul(p0[:], qbpw[:, c, :], qct[0][:, c, :],
                                     start=(c == 0), stop=(c == CQ // 128 - 1))
                    nc.tensor.matmul(p1[:], qbpw[:, c, :], qct[1][:, c, :],
                                     start=(c == 0), stop=(c == CQ // 128 - 1))
                nc.vector.tensor_mul(qp_raw[:, 0:512], p0[:], fq_bc[:, 0:512])
                nc.vector.tensor_mul(qp_raw[:, 512:1024], p1[:],
                                     fq_bc[:, 512:1024])
                emit_norm_reduce()   # prev group's partition reduces (GpSimd)
                # rope on the head-pair tile: rows [0:64]=head h0, [64:128]=h0+1
                qTp = sbh.tile([128, S], BF16, tag="qTp")
                rs = sbp.tile([128, S], F32, tag="ropes")
                for hh in range(2):
                    sl = slice(hh * 512, (hh + 1) * 512)
                    for b in range(4):
                        r0 = b * 32
                        r1 = r0 + 32 if b % 2 == 0 else r0 - 32
                        nc.vector.tensor_copy(rs[r0:r0 + 32, sl], qp_raw[r1:r1 + 32, sl])
                    nc.vector.tensor_mul(rs[:, sl], rs[:, sl], sin2tg_sb[:, sl])
                    nc.vector.tensor_mul(qp_raw[:, sl], qp_raw[:, sl], cos2t_sb[:, sl])
                    nc.vector.tensor_add(qTp[:, sl], qp_raw[:, sl], rs[:, sl])

                # --- q nope projections; st-paired so each stationary is
                # loaded once per two 512-streams ---
                qTn = []
                for i in range(G_HEADS):
                    qt_t = sbh.tile([128, S], BF16, tag=f"qTn{i}", name=f"qTn{i}")
                    p0 = ps_main.tile([128, 512], F32, tag="s", name="p0")
                    p1 = ps_main.tile([128, 512], F32, tag="s", name="p1")
                    if g == 0 and i == 0:
                        # first chains after the AllGather: unpaired, so the
                        # st=0 chain starts as soon as half the stitch lands
                        for c in range(CQ // 128):
                            nc.tensor.matmul(p0[:], qbnw[:, c, 0:128],
                                             qct[0][:, c, :],
                                             start=(c == 0), stop=(c == CQ // 128 - 1))
                        for c in range(CQ // 128):
                            nc.tensor.matmul(p1[:], qbnw[:, c, 0:128],
                                             qct[1][:, c, :],
                                             start=(c == 0), stop=(c == CQ // 128 - 1))
                    else:
                        for c in range(CQ // 128):
                            nc.tensor.matmul(p0[:], qbnw[:, c, i * 128:(i + 1) * 128],
                                             qct[0][:, c, :],
                                             start=(c == 0), stop=(c == CQ // 128 - 1))
                            nc.tensor.matmul(p1[:], qbnw[:, c, i * 128:(i + 1) * 128],
                                             qct[1][:, c, :],
                                             start=(c == 0), stop=(c == CQ // 128 - 1))
                    nc.vector.tensor_mul(qt_t[:, 0:512], p0[:], fq_bc[:, 0:512])
                    nc.vector.tensor_mul(qt_t[:, 512:1024], p1[:],
                                         fq_bc[:, 512:1024])
                    qTn.append(qt_t)
                    if i == 0:
                        emit_norm_apply()  # prev group's recip+mul (DVE)

                kT_g, v_g = kv_tiles[g]

                # --- attention: heads interleaved, AV skewed one kc behind,
                # causal windows start at the diagonal ---
                for qt in range(2):
                    kmax = 4 * (qt + 1)
                    sums = [sbs.tile([128, 512], F32, tag=f"sums{i}", name=f"sums{i}")
                            for i in range(G_HEADS)]
                    psum_o = [ps_o.tile([128, 512], F32, tag=f"o{i}", name=f"po{i}")
                              for i in range(G_HEADS)]
                    pt = {}

                    def av_step(kc, last):
                        offp = max(0, (kc - 4 * qt)) * 128
                        for i in range(G_HEADS):
                            nc.tensor.matmul(psum_o[i][:, offp:512],
                                             v_g[:, kc, i * 128:(i + 1) * 128],
                                             pt[(i, kc)][:, offp:512],
                                             start=(kc == 0), stop=last,
                                             skip_group_check=True)

                    for kc in range(kmax):
                        off = max(0, (kc - 4 * qt)) * 128
                        qsl = slice(qt * 512 + off, (qt + 1) * 512)
                        for i in range(G_HEADS):
                            ps = ps_main.tile([128, 512], F32, tag="s", name="ps")
                            nc.tensor.matmul(ps[:, off:512],
                                             kT_g[:, i, kc * 128:(kc + 1) * 128],
                                             qTn[i][:, qsl],
                                             start=True, stop=False)
                            nc.tensor.matmul(ps[:, off:512],
                                             kpe2[i * 64:(i + 1) * 64,
                                                  kc * 128:(kc + 1) * 128],
                                             qTp[i * 64:(i + 1) * 64, qsl],
                                             start=False, stop=True)
                            p = sbpt.tile([128, 512], BF16, tag="pt", name="p")
                            nc.scalar.activation(p[:, off:512], ps[:, off:512],
                                                 AF.Exp, scale=SCALE)
                            if kc >= 4 * qt:
                                nc.vector.tensor_mul(p[:, off:off + 128],
                                                     p[:, off:off + 128], tri_sb[:])
                            if kc == 0:
                                nc.vector.tensor_copy(sums[i][:], p[:])
                            else:
                                nc.vector.tensor_add(sums[i][:, off:512],
                                                     sums[i][:, off:512],
                                                     p[:, off:512])
                            pt[(i, kc)] = p
                        if kc > 0:
                            av_step(kc - 1, last=False)
                    av_step(kmax - 1, last=True)
                    for i in range(G_HEADS):
                        pending_norm.append((h0 + i, qt, sums[i], psum_o[i]))

                # produce kv for group g+2 (slides the 3-buffer window)
                if g + 2 < N_GROUPS and g + 2 >= 3:
                    kvw = load_group_kvw(g + 2)
                    for st in range(2):
                        emit_kvb_st(g + 2, st, *kvw, ckv)

            emit_norm_reduce()
            emit_norm_apply()

            # ========= Phase C: partial output projection, out^T layout =====
            for nt in range(HID // 256):
                owt = ow_pre if nt == 0 else load_ow(nt)
                for ntl in range(2):
                    pA = ps_main.tile([128, 512], F32, tag="s", name="pA")
                    pB = ps_main.tile([128, 512], F32, tag="s", name="pB")
                    for hc in range(HG):
                        lhs = owt[:, hc, ntl * 128:(ntl + 1) * 128]
                        nc.tensor.matmul(pA[:], lhs, outs_sb[:, hc, 0:512],
                                         start=(hc == 0), stop=(hc == HG - 1))
                        nc.tensor.matmul(pB[:], lhs, outs_sb[:, hc, 512:1024],
                                         start=(hc == 0), stop=(hc == HG - 1))
                    for half, pp in ((0, pA), (1, pB)):
                        osb = sbo.tile([128, 512], BF16, tag="osb", name="osb")
                        nc.scalar.copy(osb[:], pp[:])
                        nc.sync.dma_start(
                            outT.ap()[nt * 256 + ntl * 128:nt * 256 + (ntl + 1) * 128,
                                      half * 512:(half + 1) * 512], osb[:])

    nc.compile()
    return nc


def _host_inputs(hidden_states, position_ids, q_a_weight, q_a_layernorm_weight,
                 q_b_weight, kv_a_weight, kv_a_layernorm_weight, kv_b_weight,
                 o_weight):
    bf16 = ml_dtypes.bfloat16
    x = np.asarray(hidden_states, np.float32).reshape(S, HID)
    pos = np.asarray(position_ids, np.float64).reshape(S)
    q_a_w = np.asarray(q_a_weight, np.float32)
    q_ln = np.asarray(q_a_layernorm_weight, np.float32)
    q_b_w = np.asarray(q_b_weight, np.float32)
    kv_a_w = np.asarray(kv_a_weight, np.float32)
    kv_ln = np.asarray(kv_a_layernorm_weight, np.float32)
    kv_b_w = np.asarray(kv_b_weight, np.float32)
    o_w = np.asarray(o_weight, np.float32)

    wa = np.concatenate([q_a_w, kv_a_w], axis=1)               # [HID, 2112]
    xT = np.ascontiguousarray(x.T).astype(bf16)                # [HID, S]

    # per-core 320-wide wa column slabs (cores 0-6: 256 owned + 64 pad)
    slabs = np.zeros((N_CORES, HID, W_SL), np.float32)
    for c in range(7):
        slabs[c, :, 0:256] = wa[:, c * 256:(c + 1) * 256]
    slabs[7] = wa[:, 1792:2112]

    # fold the rms-norm weights into the b-projections
    qb = (q_ln[:, None] * q_b_w).reshape(CQ, H, D_Q)
    kvb = (kv_ln[:, None] * kv_b_w).reshape(CKV, H, D_NOPE + D_V)

    # rope tables
    inv_freq = 1.0 / (10000.0 ** (np.arange(0, D_ROPE, 2, dtype=np.float64) / D_ROPE))
    freqs = pos[:, None] * inv_freq[None, :]                # [S, 32]
    emb = np.concatenate([freqs, freqs], axis=-1)           # [S, 64]
    cos = np.cos(emb).astype(np.float32)
    sin = np.sin(emb).astype(np.float32)
    sin_sg = np.concatenate([-sin[:, :32], sin[:, 32:]], axis=1)  # [S, 64]
    cosT = np.ascontiguousarray(cos.T)                      # [64, S]
    sinT_sg = np.ascontiguousarray(sin_sg.T)                # [64, S]
    cos2t = np.concatenate([cosT, cosT], axis=0)            # [128, S]
    sin2tg = np.concatenate([sinT_sg, sinT_sg], axis=0)     # [128, S]
    cos_id = np.ones((D_ROPE, S), np.float32)
    sin_id = np.zeros((D_ROPE, S), np.float32)

    # causal triangle for the diagonal 128x128 blocks: key row r valid for
    # query col c iff r <= c
    i = np.arange(128)[:, None]
    j = np.arange(128)[None, :]
    tri = (i <= j).astype(np.float32).astype(bf16)

    # which cores own q columns (0-5) vs kv columns (6-7)
    coremask = np.zeros((N_CORES, 2), np.float32)
    coremask[:6, 0] = 1.0
    coremask[6:, 1] = 1.0

    in_maps = []
    for c in range(N_CORES):
        hs = slice(c * HG, (c + 1) * HG)
        in_maps.append({
            "xT": xT,
            "wsl": slabs[c].astype(bf16),
            "qbn": np.ascontiguousarray(
                qb[:, hs, :D_NOPE].reshape(CQ, HG * D_NOPE)).astype(bf16),
            "qbp": np.ascontiguousarray(
                qb[:, hs, D_NOPE:].reshape(CQ, HG * D_ROPE)).astype(bf16),
            "kvbk": np.ascontiguousarray(
                kvb[:, hs, :D_NOPE].reshape(CKV, HG * D_NOPE)).astype(bf16),
            "kvbv": np.ascontiguousarray(
                kvb[:, hs, D_NOPE:].reshape(CKV, HG * D_V)).astype(bf16),
            "ow": np.ascontiguousarray(
                o_w[c * HG * D_V:(c + 1) * HG * D_V, :]).astype(bf16),
            "cos2t": cos2t,
            "sin2tg": sin2tg,
            "cosC": cosT if c == 7 else cos_id,
            "sinC": sinT_sg if c == 7 else sin_id,
            "tri": tri,
            "coremask": coremask,
        })
    return in_maps


def kernel(**inputs):
    global LAST_EXEC_NS
    trace = bool(inputs.pop("_trace", False))
    in_maps = _host_inputs(**inputs)
    if "nc" not in _CACHE:
        _CACHE["nc"] = _build_nc()
    nc = _CACHE["nc"]
    res = bass_utils.run_bass_kernel_spmd(
        nc, in_maps, core_ids=list(range(N_CORES)), trace=trace)
    LAST_EXEC_NS = res.exec_time_ns
    total = np.zeros((HID, S), np.float64)
    for c in range(N_CORES):
        total += res.results[c]["outT"].astype(np.float64)
    return np.ascontiguousarray(total.T).astype(np.float32).reshape(1, 1, S, HID)
